# revision 1
# baseline (speedup 1.0000x reference)
"""Trainium2 Bass kernel for MarginKLDivLoss-ColBERT (retrieval maxsim + KL).

Strategy: data-parallel over batch B=128 across 8 NeuronCores (16 examples
per core). The host pre-permutes doc embeddings to [D, Ld] layout per doc so
the device streams the transposed operand for the maxsim matmul directly with
fully-contiguous DMA (layout prep only -- all FLOPs happen on device).

Per core, per example b (16), per doc j (1 pos + 8 negs):
  - mask replica [128,512] via K=1 PE matmul (ones outer mask-row)
  - masked docT  = docT * replica                     (DVE)
  - ssq per d    = ACT Square + accum_out             (ACT)
  - inv          = 1/max(sqrt(ssq),1e-12)             (ACT+DVE)
  - qs           = qnT * inv  (norm folded into q)    (DVE)
  - S [32,512]   = qs.T @ masked                      (PE)
  - maxsim col   = reduce_max(S, free)                (DVE)
Per example: sv = ones32.T @ maxvals (PE partition-sum), then the
log-softmax + KL epilogue on [1,8] tiles. Output: per-example KL sums [16];
host sums 128 values / B (the cross-device mean at gather time).
"""

import os
import sys
from contextlib import ExitStack

sys.path.insert(0, "/opt/trn_rl_repo")

import numpy as np

import concourse.bass as bass  # noqa: F401  (registers engine classes)
import concourse.bacc as bacc
import concourse.mybir as mybir
import concourse.tile as tile
from concourse.bass_utils import run_bass_kernel_spmd

N_CORES = 8
B, Lq, Ld, D, N = 128, 32, 512, 128, 8
PB = B // N_CORES  # examples per core
NDOC = N + 1

_f32 = mybir.dt.float32
_i32 = mybir.dt.int32
AF = mybir.ActivationFunctionType
ALU = mybir.AluOpType
AX = mybir.AxisListType

_PROGRAM = None
LAST_RESULTS = None


def _emit(ctx, tc, nc, aps):
    const = ctx.enter_context(tc.tile_pool(name="const", bufs=1))
    qpool = ctx.enter_context(tc.tile_pool(name="qpool", bufs=3))
    dpool = ctx.enter_context(tc.tile_pool(name="doc", bufs=6))
    mpool = ctx.enter_context(tc.tile_pool(name="masked", bufs=4))
    spool = ctx.enter_context(tc.tile_pool(name="scratch", bufs=2))
    small = ctx.enter_context(tc.tile_pool(name="small", bufs=4))
    tiny = ctx.enter_context(tc.tile_pool(name="tiny", bufs=6))
    ps_rep = ctx.enter_context(tc.tile_pool(name="ps_rep", bufs=2, space="PSUM"))
    # masks packed on partitions {0,32,64}: doc u at partition 32*(u%3), block u//3
    ps_s = ctx.enter_context(tc.tile_pool(name="ps_s", bufs=2, space="PSUM"))
    ps_q = ctx.enter_context(tc.tile_pool(name="ps_q", bufs=1, space="PSUM"))

    # Constants and whole-core loads
    ones_full = const.tile([65, D], _f32)
    nc.gpsimd.memset(ones_full[:], 1.0)
    ones32 = const.tile([Lq, 1], _f32)
    nc.gpsimd.memset(ones32[:], 1.0)
    ident = const.tile([Lq, Lq], _f32)
    nc.sync.dma_start(ident[:], aps["ident"][:])
    q_sb = const.tile([Lq, PB * D], _f32)
    nc.sync.dma_start(q_sb[:], aps["q_t"][:])
    maskp = const.tile([65, 48 * Ld], _f32)
    nc.sync.dma_start(maskp[0:1, :], aps["maskp"][0:1, :])
    nc.sync.dma_start(maskp[32:33, :], aps["maskp"][1:2, :])
    nc.sync.dma_start(maskp[64:65, :], aps["maskp"][2:3, :])
    lab_sb = const.tile([1, PB * N], _f32)
    nc.sync.dma_start(lab_sb[:], aps["labels"][:])
    out_sb = const.tile([1, PB], _f32)

    for b in range(PB):
        qb = q_sb[:, b * D : (b + 1) * D]
        qsq = spool.tile([Lq, D], _f32, tag="qsq")
        ssq_q = tiny.tile([Lq, 1], _f32, tag="ssq_q")
        nc.scalar.activation(qsq[:], qb, AF.Square, accum_out=ssq_q[:])
        nq = tiny.tile([Lq, 1], _f32, tag="nq")
        nc.scalar.activation(nq[:], ssq_q[:], AF.Sqrt)
        nqc = tiny.tile([Lq, 1], _f32, tag="nqc")
        nc.vector.tensor_scalar_max(nqc[:], nq[:], 1e-12)
        invq = tiny.tile([Lq, 1], _f32, tag="invq")
        nc.vector.reciprocal(invq[:], nqc[:])
        qn = qpool.tile([Lq, D], _f32, tag="qn")
        nc.vector.tensor_scalar_mul(qn[:], qb, invq[:])
        qnT_ps = ps_q.tile([D, Lq], _f32, tag="qnT_ps")
        # plain matmul transpose (is_transpose lowering crashes on this path)
        nc.tensor.matmul(qnT_ps[:], qn[:], ident[:], start=True, stop=True)
        qnT = qpool.tile([D, Lq], _f32, tag="qnT")
        nc.scalar.copy(qnT[:], qnT_ps[:])

        mb = small.tile([Lq, NDOC], _f32, tag="mb")
        for j in range(NDOC):
            if j == 0:
                src = aps["posT"][b]
            else:
                src = aps["negT"][(j - 1) * PB + b]
            u = b * NDOC + j
            base = 32 * (u % 3)
            blk = u // 3
            mrow = maskp[base : base + 1, blk * Ld : (blk + 1) * Ld]
            dT = dpool.tile([D, Ld], _f32, tag="dT")
            nc.sync.dma_start(dT[:], src)
            rep = ps_rep.tile([D, Ld], _f32, tag="rep")
            nc.tensor.matmul(
                rep[:], ones_full[base : base + 1, :], mrow, start=True, stop=True
            )
            masked = mpool.tile([D, Ld], _f32, tag="masked")
            nc.vector.tensor_tensor(masked[:], dT[:], rep[:], op=ALU.mult)
            msq = spool.tile([D, Ld], _f32, tag="msq")
            ssq = tiny.tile([D, 1], _f32, tag="ssq")
            nc.scalar.activation(msq[:], masked[:], AF.Square, accum_out=ssq[:])
            nrm = tiny.tile([D, 1], _f32, tag="nrm")
            nc.scalar.activation(nrm[:], ssq[:], AF.Sqrt)
            nrmc = tiny.tile([D, 1], _f32, tag="nrmc")
            nc.vector.tensor_scalar_max(nrmc[:], nrm[:], 1e-12)
            invd = tiny.tile([D, 1], _f32, tag="invd")
            nc.vector.reciprocal(invd[:], nrmc[:])
            qs = qpool.tile([D, Lq], _f32, tag="qs")
            nc.vector.tensor_scalar_mul(qs[:], qnT[:], invd[:])
            s_ps = ps_s.tile([Lq, Ld], _f32, tag="s_ps")
            nc.tensor.matmul(s_ps[:], qs[:], masked[:], start=True, stop=True)
            nc.vector.reduce_max(mb[:, j : j + 1], s_ps[:], axis=AX.X)

        sv_ps = ps_q.tile([1, NDOC], _f32, tag="sv_ps")
        nc.tensor.matmul(sv_ps[:], ones32[:], mb[:], start=True, stop=True)
        sv = small.tile([1, NDOC], _f32, tag="sv")
        nc.scalar.copy(sv[:], sv_ps[:])
        # scores d[n] = s_neg[n] - s_pos
        dsc = small.tile([1, N], _f32, tag="dsc")
        nc.vector.tensor_scalar_sub(dsc[:], sv[0:1, 1:NDOC], sv[0:1, 0:1])
        mx = tiny.tile([1, 1], _f32, tag="mx")
        nc.vector.reduce_max(mx[:], dsc[:], axis=AX.X)
        nmx = tiny.tile([1, 1], _f32, tag="nmx")
        nc.vector.tensor_scalar_mul(nmx[:], mx[:], -1.0)
        e = small.tile([1, N], _f32, tag="e")
        se = tiny.tile([1, 1], _f32, tag="se")
        nc.scalar.activation(e[:], dsc[:], AF.Exp, bias=nmx[:], accum_out=se[:])
        lse0 = tiny.tile([1, 1], _f32, tag="lse0")
        nc.scalar.activation(lse0[:], se[:], AF.Ln)
        lse = tiny.tile([1, 1], _f32, tag="lse")
        nc.vector.tensor_add(lse[:], lse0[:], mx[:])
        lab = lab_sb[0:1, b * N : (b + 1) * N]
        elab = small.tile([1, N], _f32, tag="elab")
        nc.scalar.activation(elab[:], lab, AF.Exp)
        t1 = small.tile([1, N], _f32, tag="t1")
        nc.vector.tensor_sub(t1[:], lab, dsc[:])
        t2 = small.tile([1, N], _f32, tag="t2")
        nc.vector.tensor_scalar_add(t2[:], t1[:], lse[:])
        t3 = small.tile([1, N], _f32, tag="t3")
        nc.vector.tensor_mul(t3[:], t2[:], elab[:])
        nc.vector.reduce_sum(out_sb[0:1, b : b + 1], t3[:], axis=AX.X)

    nc.sync.dma_start(aps["out"][:], out_sb[:])


def build_program():
    nc = bacc.Bacc(
        "TRN2",
        target_bir_lowering=False,
        debug=False,
        enable_asserts=True,
        num_devices=N_CORES,
    )
    aps = {
        "q_t": nc.dram_tensor("q_t", [Lq, PB * D], _f32, kind="ExternalInput").ap(),
        "posT": nc.dram_tensor("posT", [PB, D, Ld], _f32, kind="ExternalInput").ap(),
        "negT": nc.dram_tensor("negT", [N * PB, D, Ld], _f32, kind="ExternalInput").ap(),
        "maskp": nc.dram_tensor("maskp", [3, 48 * Ld], _f32, kind="ExternalInput").ap(),
        "labels": nc.dram_tensor("labels", [1, PB * N], _f32, kind="ExternalInput").ap(),
        "ident": nc.dram_tensor("ident", [Lq, Lq], _f32, kind="ExternalInput").ap(),
        "out": nc.dram_tensor("out", [1, PB], _f32, kind="ExternalOutput").ap(),
    }
    with tile.TileContext(nc) as tc:
        with ExitStack() as ctx:
            _emit(ctx, tc, nc, aps)
    nc.compile()
    return nc


def _pack_masks(pm, nm):
    """Pack the 144 per-doc masks (u = b*9 + j) so doc u's row sits at
    partition 32*(u%3), free block u//3 -- base in {0,32,64} for the PE
    replica matmul (lhsT=ones row, rhs=mask row, K=1)."""
    rows = np.zeros((144, Ld), np.float32)
    for b in range(PB):
        rows[b * NDOC] = pm[b]
        for j in range(1, NDOC):
            rows[b * NDOC + j] = nm[j - 1, b]
    out = np.zeros((3, 48 * Ld), np.float32)
    for u in range(144):
        out[u % 3, (u // 3) * Ld : (u // 3 + 1) * Ld] = rows[u]
    return out


def shard_inputs(q_emb, pos_emb, neg_emb, labels, pos_mask, neg_mask):
    q = np.ascontiguousarray(q_emb, dtype=np.float32)
    pos = np.ascontiguousarray(pos_emb, dtype=np.float32)
    neg = np.ascontiguousarray(neg_emb, dtype=np.float32)
    lab = np.ascontiguousarray(labels, dtype=np.float32)
    pm = np.ascontiguousarray(pos_mask, dtype=np.float32)
    nm = np.ascontiguousarray(neg_mask, dtype=np.float32)
    ident = np.eye(Lq, dtype=np.float32)
    in_maps = []
    for c in range(N_CORES):
        b0, b1 = c * PB, (c + 1) * PB
        in_maps.append(
            {
                "q_t": np.ascontiguousarray(
                    q[b0:b1].transpose(1, 0, 2).reshape(Lq, PB * D)
                ),
                "posT": np.ascontiguousarray(pos[b0:b1].transpose(0, 2, 1)),
                "negT": np.ascontiguousarray(
                    neg[:, b0:b1].transpose(0, 1, 3, 2).reshape(N * PB, D, Ld)
                ),
                "maskp": _pack_masks(pm[b0:b1], nm[:, b0:b1]),
                "labels": np.ascontiguousarray(lab[b0:b1].reshape(1, PB * N)),
                "ident": ident,
            }
        )
    return in_maps


def kernel(**inputs):
    global _PROGRAM, LAST_RESULTS
    if _PROGRAM is None:
        _PROGRAM = build_program()
    in_maps = shard_inputs(
        inputs["q_emb"],
        inputs["pos_emb"],
        inputs["neg_emb"],
        inputs["labels"],
        inputs["pos_mask"],
        inputs["neg_mask"],
    )
    trace = bool(int(os.environ.get("KBASS_TRACE", "0")))
    res = run_bass_kernel_spmd(
        _PROGRAM, in_maps, list(range(N_CORES)), trace=trace
    )
    LAST_RESULTS = res
    parts = np.concatenate(
        [np.asarray(res.results[c]["out"]).reshape(-1) for c in range(N_CORES)]
    )
    return np.float32(parts.sum(dtype=np.float64) / B)



# revision 7
# speedup vs baseline: 3.5822x; 3.5822x over previous
"""Trainium2 Bass kernel for MarginKLDivLoss-ColBERT (retrieval maxsim + KL).

Strategy: data-parallel over batch B=128 across 8 NeuronCores (16 examples
per core, 4 blocks of 4 examples each).

Layout prep on host (no FLOPs, pure permutation/cast/packing):
  - Each doc's 512 tokens are PERMUTED so unmasked tokens come first
    (maxsim is permutation-invariant over doc tokens); masked slots are
    zero-padded -- identical to the reference's mask-multiply zeroing.
  - Docs transposed to [D=128, Ld=512] bf16 and packed per 4-example
    block into one [128, 36*512] DMA payload (4.6 MiB -> near-peak HBM).
  - The per-doc unmasked count c becomes a compile-time prefix length
    (program specialized on the mask pattern; max over cores per slot so
    one SPMD program serves all 8 cores -- shorter docs just read some
    zero padding).

Per core, per 4-example block (36 docs):
  - ssq[d] = sum_k dT[d,0:c]^2 via ACT Square+accum (some docs) and DVE
    tensor_tensor_reduce (rest) -- split tuned to balance the engines.
  - inv = 1/max(sqrt(ssq),1e-12) batched [128,36]; folded into the
    per-doc q operand qs = qnT * inv on GPSIMD (tensor_scalar).
  - S for 4 examples' doc-j packed into one PSUM [128,512] (4 matmuls,
    full 512 cols; zero-padded tails give exact masked-token zeros).
  - One DVE reduce_max per group: [128,512] -> [128,1] (4 docs at once).
  - sv = E^T @ maxvals (E = 32-block indicator) -> [4 ex, 9 docs].
Epilogue once per core on [16,9]: log-softmax + KLDiv(log_target),
per-example sums DMA'd out; host sums 128 values / B.
"""

import os
import sys
from contextlib import ExitStack

sys.path.insert(0, "/opt/trn_rl_repo")

import ml_dtypes
import numpy as np

import concourse.bass as bass  # noqa: F401  (registers engine classes)
import concourse.bacc as bacc
import concourse.mybir as mybir
import concourse.tile as tile
from concourse.bass_utils import run_bass_kernel_spmd

N_CORES = 8
B, Lq, Ld, D, N = 128, 32, 512, 128, 8
PB = B // N_CORES          # examples per core (16)
NDOC = N + 1               # docs per example (pos + 8 negs)
NBLK = 4                   # blocks per core
BE = PB // NBLK            # examples per block (4)
BDOC = BE * NDOC           # docs per block (36)
ACT_SSQ_FRAC = 0.58        # fraction of per-doc ssq passes on ScalarE

_f32 = mybir.dt.float32
_bf16 = mybir.dt.bfloat16
_np_bf16 = ml_dtypes.bfloat16
AF = mybir.ActivationFunctionType
ALU = mybir.AluOpType
AX = mybir.AxisListType

_PROGRAM = None
_PROGRAM_KEY = None
LAST_RESULTS = None


def _emit(ctx, tc, nc, aps, c_slot):
    const = ctx.enter_context(tc.tile_pool(name="const", bufs=1))
    dpool = ctx.enter_context(tc.tile_pool(name="docs", bufs=3))
    qpool = ctx.enter_context(tc.tile_pool(name="qpool", bufs=4))
    qspool = ctx.enter_context(tc.tile_pool(name="qs", bufs=8))
    spool = ctx.enter_context(tc.tile_pool(name="scratch", bufs=2))
    small = ctx.enter_context(tc.tile_pool(name="small", bufs=4))
    tiny = ctx.enter_context(tc.tile_pool(name="tiny", bufs=8))
    ps_s = ctx.enter_context(tc.tile_pool(name="ps_s", bufs=3, space="PSUM"))
    ps_q = ctx.enter_context(tc.tile_pool(name="ps_q", bufs=2, space="PSUM"))

    # ---- constants / whole-core loads ----
    q_sb = const.tile([Lq, PB * D], _bf16)
    nc.sync.dma_start(q_sb[:], aps["q_t"][:])
    ident = const.tile([Lq, Lq], _bf16)
    nc.sync.dma_start(ident[:], aps["ident"][:])
    lab_sb = const.tile([D, N], _f32)
    nc.sync.dma_start(lab_sb[:], aps["labels"][:])
    emat = const.tile([D, BE], _f32)
    nc.gpsimd.memset(emat[:], 0.0)
    for t in range(BE):
        nc.gpsimd.memset(emat[32 * t : 32 * t + 32, t : t + 1], 1.0)
    qnT_sb = const.tile([D, PB * Lq], _bf16)
    sv_sb = const.tile([D, NDOC], _f32)
    nc.gpsimd.memset(sv_sb[:], 0.0)
    out_sb = const.tile([D, 1], _f32)

    # ---- stage A: q normalization + transpose (all 16 examples) ----
    ssq_q = const.tile([Lq, PB], _f32)
    wq = spool.tile([Lq, D], _bf16, tag="wq")
    for b in range(PB):
        qb = q_sb[:, b * D : (b + 1) * D]
        nc.scalar.activation(
            wq[:], qb, AF.Square, accum_out=ssq_q[:, b : b + 1]
        )
    nrm_q = tiny.tile([Lq, PB], _f32, tag="nrm_q")
    nc.scalar.activation(nrm_q[:], ssq_q[:], AF.Sqrt)
    nrmc_q = tiny.tile([Lq, PB], _f32, tag="nrmc_q")
    nc.vector.tensor_scalar_max(nrmc_q[:], nrm_q[:], 1e-12)
    inv_q = const.tile([Lq, PB], _f32)
    nc.vector.reciprocal(inv_q[:], nrmc_q[:])
    for b in range(PB):
        qb = q_sb[:, b * D : (b + 1) * D]
        qn = qpool.tile([Lq, D], _bf16, tag="qn")
        nc.vector.tensor_scalar_mul(qn[:], qb, inv_q[:, b : b + 1])
        qnT_ps = ps_q.tile([D, Lq], _f32, tag="qnT_ps")
        nc.tensor.matmul(qnT_ps[:], qn[:], ident[:], start=True, stop=True)
        nc.vector.tensor_copy(qnT_sb[:, b * Lq : (b + 1) * Lq], qnT_ps[:])

    # ---- stage B: main loop over 4 blocks ----
    n_act = 0
    n_tot = 0
    for blk in range(NBLK):
        dtile = dpool.tile([D, BDOC * Ld], _bf16, tag="dtile")
        nc.sync.dma_start(dtile[:], aps["docs"][blk])

        ssq = small.tile([D, BDOC], _f32, tag="ssq")
        wA = spool.tile([D, Ld], _bf16, tag="wA")
        wV = spool.tile([D, Ld], _bf16, tag="wV")
        for u in range(BDOC):
            c = c_slot[blk][u]
            if c == 0:
                nc.gpsimd.memset(ssq[:, u : u + 1], 0.0)
                continue
            seg = dtile[:, u * Ld : u * Ld + c]
            n_tot += 1
            if n_act < ACT_SSQ_FRAC * n_tot:
                n_act += 1
                nc.scalar.activation(
                    wA[:, 0:c], seg, AF.Square, accum_out=ssq[:, u : u + 1]
                )
            else:
                nc.vector.tensor_tensor(wV[:, 0:c], seg, seg, op=ALU.mult)
                nc.vector.reduce_sum(
                    ssq[:, u : u + 1], wV[:, 0:c], axis=AX.X
                )
        nrm = tiny.tile([D, BDOC], _f32, tag="nrm")
        nc.scalar.activation(nrm[:], ssq[:], AF.Sqrt)
        nrmc = tiny.tile([D, BDOC], _f32, tag="nrmc")
        nc.vector.tensor_scalar_max(nrmc[:], nrm[:], 1e-12)
        inv = tiny.tile([D, BDOC], _f32, tag="inv")
        nc.vector.reciprocal(inv[:], nrmc[:])

        maxv = small.tile([D, NDOC], _f32, tag="maxv")
        for j in range(NDOC):
            ps = ps_s.tile([D, Ld], _f32, tag="ps")
            for t in range(BE):
                u = t * NDOC + j
                b = blk * BE + t
                qs = qspool.tile([D, Lq], _bf16, tag="qs")
                nc.vector.tensor_scalar_mul(
                    qs[:], qnT_sb[:, b * Lq : (b + 1) * Lq], inv[:, u : u + 1]
                )
                nc.tensor.matmul(
                    ps[32 * t : 32 * t + 32, :],
                    qs[:],
                    dtile[:, u * Ld : (u + 1) * Ld],
                    start=True, stop=True,
                    tile_position=(0, 32 * t),
                )
            nc.vector.reduce_max(maxv[:, j : j + 1], ps[:], axis=AX.X)

        sv_ps = ps_q.tile([BE, NDOC], _f32, tag="sv_ps")
        nc.tensor.matmul(sv_ps[:], emat[:], maxv[:], start=True, stop=True)
        nc.vector.tensor_copy(sv_sb[32 * blk : 32 * blk + BE, :], sv_ps[:])

    # ---- epilogue: log-softmax + KL over [16, 9] ----
    dsc = small.tile([D, N], _f32, tag="dsc")
    nc.vector.tensor_scalar_sub(dsc[:], sv_sb[:, 1:NDOC], sv_sb[:, 0:1])
    mx = tiny.tile([D, 1], _f32, tag="mx")
    nc.vector.reduce_max(mx[:], dsc[:], axis=AX.X)
    nmx = tiny.tile([D, 1], _f32, tag="nmx")
    nc.vector.tensor_scalar_mul(nmx[:], mx[:], -1.0)
    e = small.tile([D, N], _f32, tag="e")
    se = tiny.tile([D, 1], _f32, tag="se")
    nc.scalar.activation(e[:], dsc[:], AF.Exp, bias=nmx[:], accum_out=se[:])
    lse0 = tiny.tile([D, 1], _f32, tag="lse0")
    nc.scalar.activation(lse0[:], se[:], AF.Ln)
    lse = tiny.tile([D, 1], _f32, tag="lse")
    nc.vector.tensor_add(lse[:], lse0[:], mx[:])
    elab = small.tile([D, N], _f32, tag="elab")
    nc.scalar.activation(elab[:], lab_sb[:], AF.Exp)
    t1 = small.tile([D, N], _f32, tag="t1")
    nc.vector.tensor_sub(t1[:], lab_sb[:], dsc[:])
    t2 = small.tile([D, N], _f32, tag="t2")
    nc.vector.tensor_scalar_add(t2[:], t1[:], lse[:])
    t3 = small.tile([D, N], _f32, tag="t3")
    nc.vector.tensor_mul(t3[:], t2[:], elab[:])
    nc.vector.reduce_sum(out_sb[:], t3[:], axis=AX.X)
    nc.sync.dma_start(aps["out"][:], out_sb[:])


def build_program(c_slot):
    nc = bacc.Bacc(
        "TRN2",
        target_bir_lowering=False,
        debug=False,
        enable_asserts=True,
        num_devices=N_CORES,
    )
    aps = {
        "docs": nc.dram_tensor(
            "docs", [NBLK, D, BDOC * Ld], _bf16, kind="ExternalInput"
        ).ap(),
        "q_t": nc.dram_tensor("q_t", [Lq, PB * D], _bf16, kind="ExternalInput").ap(),
        "ident": nc.dram_tensor("ident", [Lq, Lq], _bf16, kind="ExternalInput").ap(),
        "labels": nc.dram_tensor("labels", [D, N], _f32, kind="ExternalInput").ap(),
        "out": nc.dram_tensor("out", [D, 1], _f32, kind="ExternalOutput").ap(),
    }
    with tile.TileContext(nc) as tc:
        with ExitStack() as ctx:
            _emit(ctx, tc, nc, aps, c_slot)
    nc.compile()
    return nc


def shard_inputs(q_emb, pos_emb, neg_emb, labels, pos_mask, neg_mask):
    # docs_all[b, j] = j-th doc of example b (j=0 pos, j>0 neg j-1)
    docs_all = np.empty((B, NDOC, Ld, D), dtype=_np_bf16)
    docs_all[:, 0] = pos_emb.astype(_np_bf16)
    docs_all[:, 1:] = neg_emb.transpose(1, 0, 2, 3).astype(_np_bf16)
    m_all = np.empty((B, NDOC, Ld), dtype=np.int64)
    m_all[:, 0] = pos_mask
    m_all[:, 1:] = neg_mask.transpose(1, 0, 2)

    # unmasked-first token permutation (stable), zero-pad the masked tail
    order = np.argsort(1 - m_all, axis=2, kind="stable")
    c_all = m_all.sum(axis=2)  # [B, NDOC]
    gathered = np.take_along_axis(docs_all, order[..., None], axis=2)
    keep = np.arange(Ld)[None, None, :] < c_all[..., None]
    gathered[~keep] = 0

    # per-slot prefix length = max over the 8 cores (one SPMD program)
    c_by_core = c_all.reshape(N_CORES, PB, NDOC)
    c_slot = []
    for blk in range(NBLK):
        blk_c = c_by_core[:, blk * BE : (blk + 1) * BE, :]  # [8, BE, NDOC]
        c_slot.append(tuple(int(x) for x in blk_c.max(axis=0).reshape(BDOC)))
    c_slot = tuple(c_slot)

    lab = np.ascontiguousarray(labels, dtype=np.float32)
    ident = np.eye(Lq, dtype=_np_bf16)
    q_bf = q_emb.astype(_np_bf16)

    in_maps = []
    for cidx in range(N_CORES):
        b0 = cidx * PB
        # [PB, NDOC, Ld, D] -> per block [D, BE*NDOC*Ld]
        core_docs = gathered[b0 : b0 + PB]
        blocks = np.empty((NBLK, D, BDOC * Ld), dtype=_np_bf16)
        for blk in range(NBLK):
            g = core_docs[blk * BE : (blk + 1) * BE]  # [BE, NDOC, Ld, D]
            blocks[blk] = (
                g.transpose(3, 0, 1, 2).reshape(D, BDOC * Ld)
            )
        in_maps.append(
            {
                "docs": np.ascontiguousarray(blocks),
                "q_t": np.ascontiguousarray(
                    q_bf[b0 : b0 + PB].transpose(1, 0, 2).reshape(Lq, PB * D)
                ),
                "ident": ident,
                "labels": _pad_labels(lab[b0 : b0 + PB]),
            }
        )
    return in_maps, c_slot


def _pad_labels(lab_core):
    out = np.zeros((D, N), np.float32)
    for b in range(PB):
        out[32 * (b // BE) + (b % BE)] = lab_core[b]
    return out


_OUT_ROWS = np.array([32 * (b // BE) + (b % BE) for b in range(PB)])


def kernel(**inputs):
    global _PROGRAM, _PROGRAM_KEY, LAST_RESULTS
    in_maps, c_slot = shard_inputs(
        inputs["q_emb"],
        inputs["pos_emb"],
        inputs["neg_emb"],
        inputs["labels"],
        inputs["pos_mask"],
        inputs["neg_mask"],
    )
    if _PROGRAM is None or _PROGRAM_KEY != c_slot:
        _PROGRAM = build_program(c_slot)
        _PROGRAM_KEY = c_slot
    trace = bool(int(os.environ.get("KBASS_TRACE", "0")))
    res = run_bass_kernel_spmd(_PROGRAM, in_maps, list(range(N_CORES)), trace=trace)
    LAST_RESULTS = res
    parts = np.concatenate(
        [np.asarray(res.results[c]["out"]).reshape(-1)[_OUT_ROWS] for c in range(N_CORES)]
    )
    return np.float32(parts.sum(dtype=np.float64) / B)


# revision 9
# speedup vs baseline: 4.4537x; 1.2433x over previous
"""Trainium2 Bass kernel for MarginKLDivLoss-ColBERT (retrieval maxsim + KL).

Strategy: data-parallel over batch B=128 across 8 NeuronCores (16 examples
per core, 4 blocks of 4 examples each).

Layout prep on host (no FLOPs, pure permutation/cast/packing):
  - Each doc's 512 tokens are PERMUTED so unmasked tokens come first
    (maxsim is permutation-invariant over doc tokens); masked slots are
    zero-padded -- identical to the reference's mask-multiply zeroing.
  - Docs transposed to [D=128, Ld=512] bf16 and packed per 4-example
    block into one [128, 36*512] DMA payload (4.6 MiB -> near-peak HBM).
  - The per-doc unmasked count c becomes a compile-time prefix length
    (program specialized on the mask pattern; max over cores per slot so
    one SPMD program serves all 8 cores -- shorter docs just read some
    zero padding).

Per core, per 4-example block (36 docs):
  - ssq[d] = sum_k dT[d,0:c]^2 via ACT Square+accum (some docs) and DVE
    tensor_tensor_reduce (rest) -- split tuned to balance the engines.
  - inv = 1/max(sqrt(ssq),1e-12) batched [128,36]; folded into the
    per-doc q operand qs = qnT * inv on GPSIMD (tensor_scalar).
  - S for 4 examples' doc-j packed into one PSUM [128,512] (4 matmuls,
    full 512 cols; zero-padded tails give exact masked-token zeros).
  - One DVE reduce_max per group: [128,512] -> [128,1] (4 docs at once).
  - sv = E^T @ maxvals (E = 32-block indicator) -> [4 ex, 9 docs].
Epilogue once per core on [16,9]: log-softmax + KLDiv(log_target),
per-example sums DMA'd out; host sums 128 values / B.
"""

import os
import sys
from contextlib import ExitStack

sys.path.insert(0, "/opt/trn_rl_repo")

import ml_dtypes
import numpy as np

import concourse.bass as bass  # noqa: F401  (registers engine classes)
import concourse.bacc as bacc
import concourse.mybir as mybir
import concourse.tile as tile
from concourse.bass_utils import run_bass_kernel_spmd

N_CORES = 8
B, Lq, Ld, D, N = 128, 32, 512, 128, 8
PB = B // N_CORES          # examples per core (16)
NDOC = N + 1               # docs per example (pos + 8 negs)
NBLK = 4                   # blocks per core
BE = PB // NBLK            # examples per block (4)
BDOC = BE * NDOC           # docs per block (36)
K_ACT = 16                 # docs [0,K_ACT) per block do ssq on ScalarE

_f32 = mybir.dt.float32
_bf16 = mybir.dt.bfloat16
_np_bf16 = ml_dtypes.bfloat16
AF = mybir.ActivationFunctionType
ALU = mybir.AluOpType
AX = mybir.AxisListType

_PROGRAM = None
_PROGRAM_KEY = None
LAST_RESULTS = None


def _emit(ctx, tc, nc, aps, c_slot):
    const = ctx.enter_context(tc.tile_pool(name="const", bufs=1))
    dpool = ctx.enter_context(tc.tile_pool(name="docs", bufs=3))
    qpool = ctx.enter_context(tc.tile_pool(name="qpool", bufs=4))
    qspool = ctx.enter_context(tc.tile_pool(name="qs", bufs=8))
    spool = ctx.enter_context(tc.tile_pool(name="scratch", bufs=4))
    small = ctx.enter_context(tc.tile_pool(name="small", bufs=4))
    tiny = ctx.enter_context(tc.tile_pool(name="tiny", bufs=8))
    ps_s = ctx.enter_context(tc.tile_pool(name="ps_s", bufs=3, space="PSUM"))
    ps_q = ctx.enter_context(tc.tile_pool(name="ps_q", bufs=2, space="PSUM"))

    # ---- constants / whole-core loads ----
    q_sb = const.tile([Lq, PB * D], _bf16)
    nc.sync.dma_start(q_sb[:], aps["q_t"][:])
    ident = const.tile([Lq, Lq], _bf16)
    nc.sync.dma_start(ident[:], aps["ident"][:])
    lab_sb = const.tile([D, N], _f32)
    nc.sync.dma_start(lab_sb[:], aps["labels"][:])
    emat = const.tile([D, BE], _f32)
    nc.gpsimd.memset(emat[:], 0.0)
    for t in range(BE):
        nc.gpsimd.memset(emat[32 * t : 32 * t + 32, t : t + 1], 1.0)
    qnT_sb = const.tile([D, PB * Lq], _bf16)
    sv_sb = const.tile([D, NDOC], _f32)
    nc.gpsimd.memset(sv_sb[:], 0.0)
    out_sb = const.tile([D, 1], _f32)

    # ---- stage A: q normalization + transpose (all 16 examples) ----
    ssq_q = const.tile([Lq, PB], _f32)
    wq = spool.tile([Lq, D], _bf16, tag="wq")
    for b in range(PB):
        qb = q_sb[:, b * D : (b + 1) * D]
        nc.scalar.activation(
            wq[:], qb, AF.Square, accum_out=ssq_q[:, b : b + 1]
        )
    nrm_q = tiny.tile([Lq, PB], _f32, tag="nrm_q")
    nc.scalar.activation(nrm_q[:], ssq_q[:], AF.Sqrt)
    nrmc_q = tiny.tile([Lq, PB], _f32, tag="nrmc_q")
    nc.vector.tensor_scalar_max(nrmc_q[:], nrm_q[:], 1e-12)
    inv_q = const.tile([Lq, PB], _f32)
    nc.vector.reciprocal(inv_q[:], nrmc_q[:])
    for b in range(PB):
        qb = q_sb[:, b * D : (b + 1) * D]
        qn = qpool.tile([Lq, D], _bf16, tag="qn")
        nc.vector.tensor_scalar_mul(qn[:], qb, inv_q[:, b : b + 1])
        qnT_ps = ps_q.tile([D, Lq], _f32, tag="qnT_ps")
        nc.tensor.matmul(qnT_ps[:], qn[:], ident[:], start=True, stop=True)
        nc.vector.tensor_copy(qnT_sb[:, b * Lq : (b + 1) * Lq], qnT_ps[:])

    # ---- stage B: main loop over 4 blocks (groups pipelined 1 behind) ----
    cnt_sb = const.tile([D, NBLK * 2 * BDOC], _f32)
    nc.sync.dma_start(cnt_sb[:], aps["bncnt"][:])

    state = {}

    def emit_norm(blk):
        dtile = dpool.tile([D, BDOC * Ld], _bf16, tag="dtile")
        nc.sync.dma_start(dtile[:], aps["docs"][blk])
        ssq = small.tile([D, BDOC], _f32, tag="ssq")
        wA = spool.tile([D, Ld], _bf16, tag="wA")
        bnt = spool.tile([D, BDOC, 6], _f32, tag="bnt")
        k = K_ACT  # docs [0,k) on ScalarE, [k,BDOC) on DVE bn_stats
        for u in range(BDOC):
            c = c_slot[blk][u]
            if c == 0:
                continue
            seg = dtile[:, u * Ld : u * Ld + c]
            if u < k:
                nc.scalar.activation(
                    wA[:, 0:c], seg, AF.Square, accum_out=ssq[:, u : u + 1]
                )
            else:
                nc.vector.bn_stats(bnt[:, u, :], seg)
        # batched bn recovery: ssq[k:] = ve+vo + ne*me^2 + no*mo^2
        nb = BDOC - k
        ne = cnt_sb[:, blk * 2 * BDOC + k : blk * 2 * BDOC + BDOC]
        no = cnt_sb[:, blk * 2 * BDOC + BDOC + k : (blk + 1) * 2 * BDOC]
        r1 = tiny.tile([D, nb], _f32, tag="r1")
        r2 = tiny.tile([D, nb], _f32, tag="r2")
        r3 = tiny.tile([D, nb], _f32, tag="r3")
        me = bnt[:, k:BDOC, 1]
        mo = bnt[:, k:BDOC, 4]
        nc.vector.tensor_mul(r1[:], me, me)
        nc.vector.tensor_mul(r2[:], r1[:], ne)
        nc.vector.tensor_mul(r1[:], mo, mo)
        nc.vector.tensor_mul(r3[:], r1[:], no)
        nc.vector.tensor_add(r1[:], bnt[:, k:BDOC, 2], bnt[:, k:BDOC, 5])
        nc.vector.tensor_add(r2[:], r2[:], r3[:])
        nc.vector.tensor_add(ssq[:, k:BDOC], r1[:], r2[:])
        for u in range(BDOC):
            if c_slot[blk][u] == 0:
                nc.gpsimd.memset(ssq[:, u : u + 1], 0.0)
        nrm = tiny.tile([D, BDOC], _f32, tag="nrm")
        nc.scalar.activation(nrm[:], ssq[:], AF.Sqrt)
        nrmc = tiny.tile([D, BDOC], _f32, tag="nrmc")
        nc.vector.tensor_scalar_max(nrmc[:], nrm[:], 1e-12)
        inv = tiny.tile([D, BDOC], _f32, tag="inv")
        nc.vector.reciprocal(inv[:], nrmc[:])
        state[blk] = (dtile, inv)

    def emit_groups(blk):
        dtile, inv = state.pop(blk)
        maxv = small.tile([D, NDOC], _f32, tag="maxv")
        for j in range(NDOC):
            ps = ps_s.tile([D, Ld], _f32, tag="ps")
            for t in range(BE):
                u = t * NDOC + j
                b = blk * BE + t
                qs = qspool.tile([D, Lq], _bf16, tag="qs")
                nc.gpsimd.tensor_tensor(
                    qs[:],
                    qnT_sb[:, b * Lq : (b + 1) * Lq],
                    inv[:, u : u + 1].to_broadcast([D, Lq]),
                    op=ALU.mult,
                )
                nc.tensor.matmul(
                    ps[32 * t : 32 * t + 32, :],
                    qs[:],
                    dtile[:, u * Ld : (u + 1) * Ld],
                    start=True, stop=True,
                    tile_position=(0, 32 * t),
                )
            nc.vector.reduce_max(maxv[:, j : j + 1], ps[:], axis=AX.X)
        sv_ps = ps_q.tile([BE, NDOC], _f32, tag="sv_ps")
        nc.tensor.matmul(sv_ps[:], emat[:], maxv[:], start=True, stop=True)
        nc.vector.tensor_copy(sv_sb[32 * blk : 32 * blk + BE, :], sv_ps[:])

    for blk in range(NBLK):
        emit_norm(blk)
        if blk >= 1:
            emit_groups(blk - 1)
    emit_groups(NBLK - 1)

    # ---- epilogue: log-softmax + KL over [16, 9] ----
    dsc = small.tile([D, N], _f32, tag="dsc")
    nc.vector.tensor_scalar_sub(dsc[:], sv_sb[:, 1:NDOC], sv_sb[:, 0:1])
    mx = tiny.tile([D, 1], _f32, tag="mx")
    nc.vector.reduce_max(mx[:], dsc[:], axis=AX.X)
    nmx = tiny.tile([D, 1], _f32, tag="nmx")
    nc.vector.tensor_scalar_mul(nmx[:], mx[:], -1.0)
    e = small.tile([D, N], _f32, tag="e")
    se = tiny.tile([D, 1], _f32, tag="se")
    nc.scalar.activation(e[:], dsc[:], AF.Exp, bias=nmx[:], accum_out=se[:])
    lse0 = tiny.tile([D, 1], _f32, tag="lse0")
    nc.scalar.activation(lse0[:], se[:], AF.Ln)
    lse = tiny.tile([D, 1], _f32, tag="lse")
    nc.vector.tensor_add(lse[:], lse0[:], mx[:])
    elab = small.tile([D, N], _f32, tag="elab")
    nc.scalar.activation(elab[:], lab_sb[:], AF.Exp)
    t1 = small.tile([D, N], _f32, tag="t1")
    nc.vector.tensor_sub(t1[:], lab_sb[:], dsc[:])
    t2 = small.tile([D, N], _f32, tag="t2")
    nc.vector.tensor_scalar_add(t2[:], t1[:], lse[:])
    t3 = small.tile([D, N], _f32, tag="t3")
    nc.vector.tensor_mul(t3[:], t2[:], elab[:])
    nc.vector.reduce_sum(out_sb[:], t3[:], axis=AX.X)
    nc.sync.dma_start(aps["out"][:], out_sb[:])


def build_program(c_slot):
    nc = bacc.Bacc(
        "TRN2",
        target_bir_lowering=False,
        debug=False,
        enable_asserts=True,
        num_devices=N_CORES,
    )
    aps = {
        "docs": nc.dram_tensor(
            "docs", [NBLK, D, BDOC * Ld], _bf16, kind="ExternalInput"
        ).ap(),
        "q_t": nc.dram_tensor("q_t", [Lq, PB * D], _bf16, kind="ExternalInput").ap(),
        "ident": nc.dram_tensor("ident", [Lq, Lq], _bf16, kind="ExternalInput").ap(),
        "labels": nc.dram_tensor("labels", [D, N], _f32, kind="ExternalInput").ap(),
        "bncnt": nc.dram_tensor(
            "bncnt", [D, NBLK * 2 * BDOC], _f32, kind="ExternalInput"
        ).ap(),
        "out": nc.dram_tensor("out", [D, 1], _f32, kind="ExternalOutput").ap(),
    }
    with tile.TileContext(nc) as tc:
        with ExitStack() as ctx:
            _emit(ctx, tc, nc, aps, c_slot)
    nc.compile()
    return nc


def shard_inputs(q_emb, pos_emb, neg_emb, labels, pos_mask, neg_mask):
    # docs_all[b, j] = j-th doc of example b (j=0 pos, j>0 neg j-1)
    docs_all = np.empty((B, NDOC, Ld, D), dtype=_np_bf16)
    docs_all[:, 0] = pos_emb.astype(_np_bf16)
    docs_all[:, 1:] = neg_emb.transpose(1, 0, 2, 3).astype(_np_bf16)
    m_all = np.empty((B, NDOC, Ld), dtype=np.int64)
    m_all[:, 0] = pos_mask
    m_all[:, 1:] = neg_mask.transpose(1, 0, 2)

    # unmasked-first token permutation (stable), zero-pad the masked tail
    order = np.argsort(1 - m_all, axis=2, kind="stable")
    c_all = m_all.sum(axis=2)  # [B, NDOC]
    gathered = np.take_along_axis(docs_all, order[..., None], axis=2)
    keep = np.arange(Ld)[None, None, :] < c_all[..., None]
    gathered[~keep] = 0

    # per-slot prefix length = max over the 8 cores (one SPMD program)
    c_by_core = c_all.reshape(N_CORES, PB, NDOC)
    c_slot = []
    for blk in range(NBLK):
        blk_c = c_by_core[:, blk * BE : (blk + 1) * BE, :]  # [8, BE, NDOC]
        c_slot.append(tuple(int(x) for x in blk_c.max(axis=0).reshape(BDOC)))
    c_slot = tuple(c_slot)

    bncnt = np.zeros((D, NBLK * 2 * BDOC), np.float32)
    for blk in range(NBLK):
        for u in range(BDOC):
            c = c_slot[blk][u]
            bncnt[:, blk * 2 * BDOC + u] = (c + 1) // 2
            bncnt[:, blk * 2 * BDOC + BDOC + u] = c // 2
    lab = np.ascontiguousarray(labels, dtype=np.float32)
    ident = np.eye(Lq, dtype=_np_bf16)
    q_bf = q_emb.astype(_np_bf16)

    in_maps = []
    for cidx in range(N_CORES):
        b0 = cidx * PB
        # [PB, NDOC, Ld, D] -> per block [D, BE*NDOC*Ld]
        core_docs = gathered[b0 : b0 + PB]
        blocks = np.empty((NBLK, D, BDOC * Ld), dtype=_np_bf16)
        for blk in range(NBLK):
            g = core_docs[blk * BE : (blk + 1) * BE]  # [BE, NDOC, Ld, D]
            blocks[blk] = (
                g.transpose(3, 0, 1, 2).reshape(D, BDOC * Ld)
            )
        in_maps.append(
            {
                "docs": np.ascontiguousarray(blocks),
                "q_t": np.ascontiguousarray(
                    q_bf[b0 : b0 + PB].transpose(1, 0, 2).reshape(Lq, PB * D)
                ),
                "ident": ident,
                "labels": _pad_labels(lab[b0 : b0 + PB]),
                "bncnt": bncnt,
            }
        )
    return in_maps, c_slot


def _pad_labels(lab_core):
    out = np.zeros((D, N), np.float32)
    for b in range(PB):
        out[32 * (b // BE) + (b % BE)] = lab_core[b]
    return out


_OUT_ROWS = np.array([32 * (b // BE) + (b % BE) for b in range(PB)])


def kernel(**inputs):
    global _PROGRAM, _PROGRAM_KEY, LAST_RESULTS
    in_maps, c_slot = shard_inputs(
        inputs["q_emb"],
        inputs["pos_emb"],
        inputs["neg_emb"],
        inputs["labels"],
        inputs["pos_mask"],
        inputs["neg_mask"],
    )
    if _PROGRAM is None or _PROGRAM_KEY != c_slot:
        _PROGRAM = build_program(c_slot)
        _PROGRAM_KEY = c_slot
    trace = bool(int(os.environ.get("KBASS_TRACE", "0")))
    res = run_bass_kernel_spmd(_PROGRAM, in_maps, list(range(N_CORES)), trace=trace)
    LAST_RESULTS = res
    parts = np.concatenate(
        [np.asarray(res.results[c]["out"]).reshape(-1)[_OUT_ROWS] for c in range(N_CORES)]
    )
    return np.float32(parts.sum(dtype=np.float64) / B)


# revision 10
# speedup vs baseline: 4.6162x; 1.0365x over previous
"""Trainium2 Bass kernel for MarginKLDivLoss-ColBERT (retrieval maxsim + KL).

Strategy: data-parallel over batch B=128 across 8 NeuronCores (16 examples
per core, 4 blocks of 4 examples each).

Layout prep on host (no FLOPs, pure permutation/cast/packing):
  - Each doc's 512 tokens are PERMUTED so unmasked tokens come first
    (maxsim is permutation-invariant over doc tokens); masked slots are
    zero-padded -- identical to the reference's mask-multiply zeroing.
  - Docs transposed to [D=128, Ld=512] bf16, one contiguous 1.15 MiB DMA
    per example.
  - The per-doc unmasked count c becomes a compile-time prefix length
    (program specialized on the mask pattern; max over cores per slot so
    one SPMD program serves all 8 cores -- shorter docs just read some
    zero padding).

Per core, per 4-example block (36 docs, norm columns in j-major order
u = j*BE + t):
  - ssq[d] = sum_k dT[d,0:c]^2: first K_ACT docs via ACT Square+accum,
    rest via one DVE bn_stats each + a batched moment-recovery
    (ssq = M2e + ne*me^2 + M2o + no*mo^2) over the remaining columns.
  - inv = 1/max(sqrt(ssq),1e-12) batched [128,36].
  - Per group j: ONE batched GPSIMD fold qs4 = qnT(4 examples) * inv
    (free-broadcast), then 4 matmuls into one PSUM [128,512] (quadrants
    0/32/64/96), full 512 cols (zero-padded tails = exact masked zeros).
  - One DVE reduce_max per group: [128,512] -> [128,1] (4 docs at once).
  - sv = E^T @ maxvals (E = 32-block indicator) -> [4 ex, 9 docs].
Groups are emitted one block behind the norm pass so DVE/ACT (norm of
block k+1) overlaps PE/GPSIMD (groups of block k).
Epilogue once per core: log-softmax + KLDiv(log_target) on scattered
rows 32*blk+t; per-example sums DMA'd out; host sums 128 values / B.
"""

import os
import sys
from contextlib import ExitStack

sys.path.insert(0, "/opt/trn_rl_repo")

import ml_dtypes
import numpy as np

import concourse.bass as bass  # noqa: F401  (registers engine classes)
import concourse.bacc as bacc
import concourse.mybir as mybir
import concourse.tile as tile
from concourse.bass_utils import run_bass_kernel_spmd

N_CORES = 8
B, Lq, Ld, D, N = 128, 32, 512, 128, 8
PB = B // N_CORES          # examples per core (16)
NDOC = N + 1               # docs per example (pos + 8 negs)
NBLK = 4                   # blocks per core
BE = PB // NBLK            # examples per block (4)
BDOC = BE * NDOC           # docs per block (36)
K_ACT = 15                 # norm columns [0,K_ACT) per block on ScalarE

_f32 = mybir.dt.float32
_bf16 = mybir.dt.bfloat16
_np_bf16 = ml_dtypes.bfloat16
AF = mybir.ActivationFunctionType
ALU = mybir.AluOpType
AX = mybir.AxisListType

_PROGRAM = None
_PROGRAM_KEY = None
LAST_RESULTS = None


def _emit(ctx, tc, nc, aps, c_slot):
    const = ctx.enter_context(tc.tile_pool(name="const", bufs=1))
    dpool = ctx.enter_context(tc.tile_pool(name="docs", bufs=10))
    qpool = ctx.enter_context(tc.tile_pool(name="qpool", bufs=4))
    qspool = ctx.enter_context(tc.tile_pool(name="qs", bufs=6))
    spool = ctx.enter_context(tc.tile_pool(name="scratch", bufs=4))
    small = ctx.enter_context(tc.tile_pool(name="small", bufs=4))
    tiny = ctx.enter_context(tc.tile_pool(name="tiny", bufs=8))
    ps_s = ctx.enter_context(tc.tile_pool(name="ps_s", bufs=3, space="PSUM"))
    ps_q = ctx.enter_context(tc.tile_pool(name="ps_q", bufs=2, space="PSUM"))

    # ---- constants / whole-core loads ----
    q_sb = const.tile([Lq, PB * D], _bf16)
    nc.sync.dma_start(q_sb[:], aps["q_t"][:])
    ident = const.tile([Lq, Lq], _bf16)
    nc.sync.dma_start(ident[:], aps["ident"][:])
    lab_sb = const.tile([D, N], _f32)
    nc.sync.dma_start(lab_sb[:], aps["labels"][:])
    cnt_sb = const.tile([D, NBLK * 2 * BDOC], _f32)
    nc.sync.dma_start(cnt_sb[:], aps["bncnt"][:])
    emat = const.tile([D, BE], _f32)
    nc.gpsimd.memset(emat[:], 0.0)
    for t in range(BE):
        nc.gpsimd.memset(emat[32 * t : 32 * t + 32, t : t + 1], 1.0)
    qnT_sb = const.tile([D, PB * Lq], _bf16)
    sv_sb = const.tile([D, NDOC], _f32)
    nc.gpsimd.memset(sv_sb[:], 0.0)
    out_sb = const.tile([D, 1], _f32)

    # ---- stage A: q normalization + transpose (all 16 examples) ----
    # ssq_q via DVE bn_stats (c=D=128 -> ne=no=64), batched recovery.
    bnq = const.tile([Lq, PB, 6], _f32)
    for b in range(PB):
        nc.vector.bn_stats(bnq[:, b, :], q_sb[:, b * D : (b + 1) * D])
    ssq_q = const.tile([Lq, PB], _f32)
    rq1 = tiny.tile([Lq, PB], _f32, tag="rq1")
    rq2 = tiny.tile([Lq, PB], _f32, tag="rq2")
    nc.vector.tensor_mul(rq1[:], bnq[:, :, 1], bnq[:, :, 1])
    nc.vector.tensor_mul(rq2[:], bnq[:, :, 4], bnq[:, :, 4])
    nc.vector.tensor_add(rq1[:], rq1[:], rq2[:])
    nc.vector.tensor_scalar_mul(rq1[:], rq1[:], float(D // 2))
    nc.vector.tensor_add(rq2[:], bnq[:, :, 2], bnq[:, :, 5])
    nc.vector.tensor_add(ssq_q[:], rq1[:], rq2[:])
    nrm_q = tiny.tile([Lq, PB], _f32, tag="nrm_q")
    nc.scalar.activation(nrm_q[:], ssq_q[:], AF.Sqrt)
    nrmc_q = tiny.tile([Lq, PB], _f32, tag="nrmc_q")
    nc.vector.tensor_scalar_max(nrmc_q[:], nrm_q[:], 1e-12)
    inv_q = const.tile([Lq, PB], _f32)
    nc.vector.reciprocal(inv_q[:], nrmc_q[:])
    for b in range(PB):
        qb = q_sb[:, b * D : (b + 1) * D]
        qn = qpool.tile([Lq, D], _bf16, tag="qn")
        nc.vector.tensor_scalar_mul(qn[:], qb, inv_q[:, b : b + 1])
        qnT_ps = ps_q.tile([D, Lq], _f32, tag="qnT_ps")
        nc.tensor.matmul(qnT_ps[:], qn[:], ident[:], start=True, stop=True)
        nc.vector.tensor_copy(qnT_sb[:, b * Lq : (b + 1) * Lq], qnT_ps[:])

    # ---- stage B: norm pass per block; groups pipelined 1 block behind ----
    state = {}

    def emit_norm(blk):
        dts = []
        for t in range(BE):
            dt = dpool.tile([D, NDOC * Ld], _bf16, tag="dt")
            nc.sync.dma_start(dt[:], aps["docs"][blk * BE + t])
            dts.append(dt)
        ssq = small.tile([D, BDOC], _f32, tag="ssq")
        wA = spool.tile([D, Ld], _bf16, tag="wA")
        bnt = spool.tile([D, BDOC, 6], _f32, tag="bnt")
        for u in range(BDOC):
            j, t = u // BE, u % BE
            c = c_slot[blk][u]
            if c == 0:
                continue
            seg = dts[t][:, j * Ld : j * Ld + c]
            if u < K_ACT:
                nc.scalar.activation(
                    wA[:, 0:c], seg, AF.Square, accum_out=ssq[:, u : u + 1]
                )
            else:
                nc.vector.bn_stats(bnt[:, u, :], seg)
        # batched bn recovery: ssq[K_ACT:] = M2e+M2o + ne*me^2 + no*mo^2
        nb = BDOC - K_ACT
        ne = cnt_sb[:, blk * 2 * BDOC + K_ACT : blk * 2 * BDOC + BDOC]
        no = cnt_sb[:, blk * 2 * BDOC + BDOC + K_ACT : (blk + 1) * 2 * BDOC]
        r1 = tiny.tile([D, nb], _f32, tag="r1")
        r2 = tiny.tile([D, nb], _f32, tag="r2")
        r3 = tiny.tile([D, nb], _f32, tag="r3")
        nc.vector.tensor_mul(r1[:], bnt[:, K_ACT:BDOC, 1], bnt[:, K_ACT:BDOC, 1])
        nc.vector.tensor_mul(r2[:], r1[:], ne)
        nc.vector.tensor_mul(r1[:], bnt[:, K_ACT:BDOC, 4], bnt[:, K_ACT:BDOC, 4])
        nc.vector.tensor_mul(r3[:], r1[:], no)
        nc.vector.tensor_add(r1[:], bnt[:, K_ACT:BDOC, 2], bnt[:, K_ACT:BDOC, 5])
        nc.vector.tensor_add(r2[:], r2[:], r3[:])
        nc.vector.tensor_add(ssq[:, K_ACT:BDOC], r1[:], r2[:])
        for u in range(BDOC):
            if c_slot[blk][u] == 0:
                nc.gpsimd.memset(ssq[:, u : u + 1], 0.0)
        nrm = tiny.tile([D, BDOC], _f32, tag="nrm")
        nc.scalar.activation(nrm[:], ssq[:], AF.Sqrt)
        nrmc = tiny.tile([D, BDOC], _f32, tag="nrmc")
        nc.vector.tensor_scalar_max(nrmc[:], nrm[:], 1e-12)
        inv = tiny.tile([D, BDOC], _f32, tag="inv")
        nc.vector.reciprocal(inv[:], nrmc[:])
        state[blk] = (dts, inv)

    def emit_groups(blk):
        dts, inv = state.pop(blk)
        qnT4 = qnT_sb[:, blk * BE * Lq : (blk + 1) * BE * Lq]
        maxv = small.tile([D, NDOC], _f32, tag="maxv")
        for j in range(NDOC):
            qs4 = qspool.tile([D, BE * Lq], _bf16, tag="qs4")
            nc.gpsimd.tensor_tensor(
                qs4[:],
                qnT4,
                inv[:, j * BE : (j + 1) * BE].to_broadcast([D, BE, Lq]),
                op=ALU.mult,
            )
            ps = ps_s.tile([D, Ld], _f32, tag="ps")
            for t in range(BE):
                nc.tensor.matmul(
                    ps[32 * t : 32 * t + 32, :],
                    qs4[:, t * Lq : (t + 1) * Lq],
                    dts[t][:, j * Ld : (j + 1) * Ld],
                    start=True, stop=True,
                    tile_position=(0, 32 * t),
                )
            nc.vector.reduce_max(maxv[:, j : j + 1], ps[:], axis=AX.X)
        sv_ps = ps_q.tile([BE, NDOC], _f32, tag="sv_ps")
        nc.tensor.matmul(sv_ps[:], emat[:], maxv[:], start=True, stop=True)
        nc.vector.tensor_copy(sv_sb[32 * blk : 32 * blk + BE, :], sv_ps[:])

    for blk in range(NBLK):
        emit_norm(blk)
        if blk >= 1:
            emit_groups(blk - 1)
    emit_groups(NBLK - 1)

    # ---- epilogue: log-softmax + KL, rows 32*blk+t ----
    dsc = small.tile([D, N], _f32, tag="dsc")
    nc.vector.tensor_scalar_sub(dsc[:], sv_sb[:, 1:NDOC], sv_sb[:, 0:1])
    mx = tiny.tile([D, 1], _f32, tag="mx")
    nc.vector.reduce_max(mx[:], dsc[:], axis=AX.X)
    nmx = tiny.tile([D, 1], _f32, tag="nmx")
    nc.vector.tensor_scalar_mul(nmx[:], mx[:], -1.0)
    e = small.tile([D, N], _f32, tag="e")
    se = tiny.tile([D, 1], _f32, tag="se")
    nc.scalar.activation(e[:], dsc[:], AF.Exp, bias=nmx[:], accum_out=se[:])
    lse0 = tiny.tile([D, 1], _f32, tag="lse0")
    nc.scalar.activation(lse0[:], se[:], AF.Ln)
    lse = tiny.tile([D, 1], _f32, tag="lse")
    nc.vector.tensor_add(lse[:], lse0[:], mx[:])
    elab = small.tile([D, N], _f32, tag="elab")
    nc.scalar.activation(elab[:], lab_sb[:], AF.Exp)
    t1 = small.tile([D, N], _f32, tag="t1")
    nc.vector.tensor_sub(t1[:], lab_sb[:], dsc[:])
    t2 = small.tile([D, N], _f32, tag="t2")
    nc.vector.tensor_scalar_add(t2[:], t1[:], lse[:])
    t3 = small.tile([D, N], _f32, tag="t3")
    nc.vector.tensor_mul(t3[:], t2[:], elab[:])
    nc.vector.reduce_sum(out_sb[:], t3[:], axis=AX.X)
    nc.sync.dma_start(aps["out"][:], out_sb[:])


def build_program(c_slot):
    nc = bacc.Bacc(
        "TRN2",
        target_bir_lowering=False,
        debug=False,
        enable_asserts=True,
        num_devices=N_CORES,
    )
    aps = {
        "docs": nc.dram_tensor(
            "docs", [PB, D, NDOC * Ld], _bf16, kind="ExternalInput"
        ).ap(),
        "q_t": nc.dram_tensor("q_t", [Lq, PB * D], _bf16, kind="ExternalInput").ap(),
        "ident": nc.dram_tensor("ident", [Lq, Lq], _bf16, kind="ExternalInput").ap(),
        "labels": nc.dram_tensor("labels", [D, N], _f32, kind="ExternalInput").ap(),
        "bncnt": nc.dram_tensor(
            "bncnt", [D, NBLK * 2 * BDOC], _f32, kind="ExternalInput"
        ).ap(),
        "out": nc.dram_tensor("out", [D, 1], _f32, kind="ExternalOutput").ap(),
    }
    with tile.TileContext(nc) as tc:
        with ExitStack() as ctx:
            _emit(ctx, tc, nc, aps, c_slot)
    nc.compile()
    return nc


def shard_inputs(q_emb, pos_emb, neg_emb, labels, pos_mask, neg_mask):
    # docs_all[b, j] = j-th doc of example b (j=0 pos, j>0 neg j-1)
    docs_all = np.empty((B, NDOC, Ld, D), dtype=_np_bf16)
    docs_all[:, 0] = pos_emb.astype(_np_bf16)
    docs_all[:, 1:] = neg_emb.transpose(1, 0, 2, 3).astype(_np_bf16)
    m_all = np.empty((B, NDOC, Ld), dtype=np.int64)
    m_all[:, 0] = pos_mask
    m_all[:, 1:] = neg_mask.transpose(1, 0, 2)

    # unmasked-first token permutation (stable), zero-pad the masked tail
    order = np.argsort(1 - m_all, axis=2, kind="stable")
    c_all = m_all.sum(axis=2)  # [B, NDOC]
    gathered = np.take_along_axis(docs_all, order[..., None], axis=2)
    keep = np.arange(Ld)[None, None, :] < c_all[..., None]
    gathered[~keep] = 0

    # per-slot prefix length = max over the 8 cores (one SPMD program);
    # norm-column order is j-major: u = j*BE + t
    c_by_core = c_all.reshape(N_CORES, PB, NDOC)
    c_slot = []
    for blk in range(NBLK):
        blk_c = c_by_core[:, blk * BE : (blk + 1) * BE, :].max(axis=0)  # [BE, NDOC]
        c_slot.append(tuple(int(blk_c[t, j]) for j in range(NDOC) for t in range(BE)))
    c_slot = tuple(c_slot)

    bncnt = np.zeros((D, NBLK * 2 * BDOC), np.float32)
    for blk in range(NBLK):
        for u in range(BDOC):
            c = c_slot[blk][u]
            bncnt[:, blk * 2 * BDOC + u] = (c + 1) // 2
            bncnt[:, blk * 2 * BDOC + BDOC + u] = c // 2
    lab = np.ascontiguousarray(labels, dtype=np.float32)
    ident = np.eye(Lq, dtype=_np_bf16)
    q_bf = q_emb.astype(_np_bf16)

    in_maps = []
    for cidx in range(N_CORES):
        b0 = cidx * PB
        core_docs = gathered[b0 : b0 + PB]  # [PB, NDOC, Ld, D]
        docs_t = np.ascontiguousarray(
            core_docs.transpose(0, 3, 1, 2).reshape(PB, D, NDOC * Ld)
        )
        in_maps.append(
            {
                "docs": docs_t,
                "q_t": np.ascontiguousarray(
                    q_bf[b0 : b0 + PB].transpose(1, 0, 2).reshape(Lq, PB * D)
                ),
                "ident": ident,
                "labels": _pad_labels(lab[b0 : b0 + PB]),
                "bncnt": bncnt,
            }
        )
    return in_maps, c_slot


def _pad_labels(lab_core):
    out = np.zeros((D, N), np.float32)
    for b in range(PB):
        out[32 * (b // BE) + (b % BE)] = lab_core[b]
    return out


_OUT_ROWS = np.array([32 * (b // BE) + (b % BE) for b in range(PB)])


def kernel(**inputs):
    global _PROGRAM, _PROGRAM_KEY, LAST_RESULTS
    in_maps, c_slot = shard_inputs(
        inputs["q_emb"],
        inputs["pos_emb"],
        inputs["neg_emb"],
        inputs["labels"],
        inputs["pos_mask"],
        inputs["neg_mask"],
    )
    if _PROGRAM is None or _PROGRAM_KEY != c_slot:
        _PROGRAM = build_program(c_slot)
        _PROGRAM_KEY = c_slot
    trace = bool(int(os.environ.get("KBASS_TRACE", "0")))
    res = run_bass_kernel_spmd(_PROGRAM, in_maps, list(range(N_CORES)), trace=trace)
    LAST_RESULTS = res
    parts = np.concatenate(
        [np.asarray(res.results[c]["out"]).reshape(-1)[_OUT_ROWS] for c in range(N_CORES)]
    )
    return np.float32(parts.sum(dtype=np.float64) / B)


# revision 11
# speedup vs baseline: 4.6794x; 1.0137x over previous
"""Trainium2 Bass kernel for MarginKLDivLoss-ColBERT (retrieval maxsim + KL).

Strategy: data-parallel over batch B=128 across 8 NeuronCores (16 examples
per core, 4 blocks of 4 examples each).

Layout prep on host (no FLOPs, pure permutation/cast/packing):
  - Each doc's 512 tokens are PERMUTED so unmasked tokens come first
    (maxsim is permutation-invariant over doc tokens); masked slots are
    zero-padded -- identical to the reference's mask-multiply zeroing.
  - Docs transposed to [D=128, Ld=512] bf16, one contiguous 1.15 MiB DMA
    per example.
  - The per-doc unmasked count c becomes a compile-time prefix length
    (program specialized on the mask pattern; max over cores per slot so
    one SPMD program serves all 8 cores -- shorter docs just read some
    zero padding).

Per core, per 4-example block (36 docs, norm columns in j-major order
u = j*BE + t):
  - ssq[d] = sum_k dT[d,0:c]^2: first K_ACT docs via ACT Square+accum,
    rest via one DVE bn_stats each + a batched moment-recovery
    (ssq = M2e + ne*me^2 + M2o + no*mo^2) over the remaining columns.
  - inv = 1/max(sqrt(ssq),1e-12) batched [128,36].
  - Per group j: ONE batched GPSIMD fold qs4 = qnT(4 examples) * inv
    (free-broadcast), then 4 matmuls into one PSUM [128,512] (quadrants
    0/32/64/96), full 512 cols (zero-padded tails = exact masked zeros).
  - One DVE reduce_max per group: [128,512] -> [128,1] (4 docs at once).
  - sv = E^T @ maxvals (E = 32-block indicator) -> [4 ex, 9 docs].
Groups are emitted one block behind the norm pass so DVE/ACT (norm of
block k+1) overlaps PE/GPSIMD (groups of block k).
Epilogue once per core: log-softmax + KLDiv(log_target) on scattered
rows 32*blk+t; per-example sums DMA'd out; host sums 128 values / B.
"""

import os
import sys
from contextlib import ExitStack

sys.path.insert(0, "/opt/trn_rl_repo")

import ml_dtypes
import numpy as np

import concourse.bass as bass  # noqa: F401  (registers engine classes)
import concourse.bacc as bacc
import concourse.mybir as mybir
import concourse.tile as tile
from concourse.bass_utils import run_bass_kernel_spmd

N_CORES = 8
B, Lq, Ld, D, N = 128, 32, 512, 128, 8
PB = B // N_CORES          # examples per core (16)
NDOC = N + 1               # docs per example (pos + 8 negs)
NBLK = 4                   # blocks per core
BE = PB // NBLK            # examples per block (4)
BDOC = BE * NDOC           # docs per block (36)
K_ACT = 19                 # norm columns [0,K_ACT) per block on ScalarE

_f32 = mybir.dt.float32
_bf16 = mybir.dt.bfloat16
_np_bf16 = ml_dtypes.bfloat16
AF = mybir.ActivationFunctionType
ALU = mybir.AluOpType
AX = mybir.AxisListType

_PROGRAM = None
_PROGRAM_KEY = None
LAST_RESULTS = None


def _emit(ctx, tc, nc, aps, c_slot):
    const = ctx.enter_context(tc.tile_pool(name="const", bufs=1))
    dpool = ctx.enter_context(tc.tile_pool(name="docs", bufs=17))
    qpool = ctx.enter_context(tc.tile_pool(name="qpool", bufs=4))
    qspool = ctx.enter_context(tc.tile_pool(name="qs", bufs=6))
    spool = ctx.enter_context(tc.tile_pool(name="scratch", bufs=4))
    small = ctx.enter_context(tc.tile_pool(name="small", bufs=4))
    tiny = ctx.enter_context(tc.tile_pool(name="tiny", bufs=8))
    ps_s = ctx.enter_context(tc.tile_pool(name="ps_s", bufs=3, space="PSUM"))
    ps_q = ctx.enter_context(tc.tile_pool(name="ps_q", bufs=2, space="PSUM"))

    # ---- constants / whole-core loads ----
    q_sb = const.tile([Lq, PB * D], _bf16)
    nc.sync.dma_start(q_sb[:], aps["q_t"][:])
    ident = const.tile([Lq, Lq], _bf16)
    nc.sync.dma_start(ident[:], aps["ident"][:])
    lab_sb = const.tile([D, N], _f32)
    nc.sync.dma_start(lab_sb[:], aps["labels"][:])
    cnt_sb = const.tile([D, NBLK * 2 * BDOC], _f32)
    nc.sync.dma_start(cnt_sb[:], aps["bncnt"][:])
    emat = const.tile([D, BE], _f32)
    nc.gpsimd.memset(emat[:], 0.0)
    for t in range(BE):
        nc.gpsimd.memset(emat[32 * t : 32 * t + 32, t : t + 1], 1.0)
    qnT_sb = const.tile([D, PB * Lq], _bf16)
    sv_sb = const.tile([D, NDOC], _f32)
    nc.gpsimd.memset(sv_sb[:], 0.0)
    out_sb = const.tile([D, 1], _f32)

    # ---- stage A: q normalization + transpose (all 16 examples) ----
    # ssq_q via DVE bn_stats (c=D=128 -> ne=no=64), batched recovery.
    bnq = const.tile([Lq, PB, 6], _f32)
    for b in range(PB):
        nc.vector.bn_stats(bnq[:, b, :], q_sb[:, b * D : (b + 1) * D])
    ssq_q = const.tile([Lq, PB], _f32)
    rq1 = tiny.tile([Lq, PB], _f32, tag="rq1")
    rq2 = tiny.tile([Lq, PB], _f32, tag="rq2")
    nc.vector.tensor_mul(rq1[:], bnq[:, :, 1], bnq[:, :, 1])
    nc.vector.tensor_mul(rq2[:], bnq[:, :, 4], bnq[:, :, 4])
    nc.vector.tensor_add(rq1[:], rq1[:], rq2[:])
    nc.vector.tensor_scalar_mul(rq1[:], rq1[:], float(D // 2))
    nc.vector.tensor_add(rq2[:], bnq[:, :, 2], bnq[:, :, 5])
    nc.vector.tensor_add(ssq_q[:], rq1[:], rq2[:])
    nrm_q = tiny.tile([Lq, PB], _f32, tag="nrm_q")
    nc.scalar.activation(nrm_q[:], ssq_q[:], AF.Sqrt)
    nrmc_q = tiny.tile([Lq, PB], _f32, tag="nrmc_q")
    nc.vector.tensor_scalar_max(nrmc_q[:], nrm_q[:], 1e-12)
    inv_q = const.tile([Lq, PB], _f32)
    nc.vector.reciprocal(inv_q[:], nrmc_q[:])
    for b in range(PB):
        qb = q_sb[:, b * D : (b + 1) * D]
        qn = qpool.tile([Lq, D], _bf16, tag="qn")
        nc.vector.tensor_scalar_mul(qn[:], qb, inv_q[:, b : b + 1])
        qnT_ps = ps_q.tile([D, Lq], _f32, tag="qnT_ps")
        nc.tensor.matmul(qnT_ps[:], qn[:], ident[:], start=True, stop=True)
        nc.vector.tensor_copy(qnT_sb[:, b * Lq : (b + 1) * Lq], qnT_ps[:])

    # ---- stage B: norm pass per block; groups pipelined 1 block behind ----
    state = {}

    def emit_norm(blk):
        dts = []
        for t in range(BE):
            dt = dpool.tile([D, NDOC * Ld], _bf16, tag="dt")
            nc.sync.dma_start(dt[:], aps["docs"][blk * BE + t])
            dts.append(dt)
        ssq = small.tile([D, BDOC], _f32, tag="ssq")
        wA = spool.tile([D, Ld], _bf16, tag="wA")
        bnt = spool.tile([D, BDOC, 6], _f32, tag="bnt")
        for u in range(BDOC):
            j, t = u // BE, u % BE
            c = c_slot[blk][u]
            if c == 0:
                continue
            seg = dts[t][:, j * Ld : j * Ld + c]
            if u < K_ACT:
                nc.scalar.activation(
                    wA[:, 0:c], seg, AF.Square, accum_out=ssq[:, u : u + 1]
                )
            else:
                nc.vector.bn_stats(bnt[:, u, :], seg)
        # batched bn recovery: ssq[K_ACT:] = M2e+M2o + ne*me^2 + no*mo^2
        nb = BDOC - K_ACT
        ne = cnt_sb[:, blk * 2 * BDOC + K_ACT : blk * 2 * BDOC + BDOC]
        no = cnt_sb[:, blk * 2 * BDOC + BDOC + K_ACT : (blk + 1) * 2 * BDOC]
        r1 = tiny.tile([D, nb], _f32, tag="r1")
        r2 = tiny.tile([D, nb], _f32, tag="r2")
        r3 = tiny.tile([D, nb], _f32, tag="r3")
        nc.vector.tensor_mul(r1[:], bnt[:, K_ACT:BDOC, 1], bnt[:, K_ACT:BDOC, 1])
        nc.vector.tensor_mul(r2[:], r1[:], ne)
        nc.vector.tensor_mul(r1[:], bnt[:, K_ACT:BDOC, 4], bnt[:, K_ACT:BDOC, 4])
        nc.vector.tensor_mul(r3[:], r1[:], no)
        nc.vector.tensor_add(r1[:], bnt[:, K_ACT:BDOC, 2], bnt[:, K_ACT:BDOC, 5])
        nc.vector.tensor_add(r2[:], r2[:], r3[:])
        nc.vector.tensor_add(ssq[:, K_ACT:BDOC], r1[:], r2[:])
        for u in range(BDOC):
            if c_slot[blk][u] == 0:
                nc.gpsimd.memset(ssq[:, u : u + 1], 0.0)
        nrm = tiny.tile([D, BDOC], _f32, tag="nrm")
        nc.scalar.activation(nrm[:], ssq[:], AF.Sqrt)
        nrmc = tiny.tile([D, BDOC], _f32, tag="nrmc")
        nc.vector.tensor_scalar_max(nrmc[:], nrm[:], 1e-12)
        inv = tiny.tile([D, BDOC], _f32, tag="inv")
        nc.vector.reciprocal(inv[:], nrmc[:])
        state[blk] = (dts, inv)

    def emit_groups(blk):
        dts, inv = state.pop(blk)
        qnT4 = qnT_sb[:, blk * BE * Lq : (blk + 1) * BE * Lq]
        maxv = small.tile([D, NDOC], _f32, tag="maxv")
        for j in range(NDOC):
            qs4 = qspool.tile([D, BE * Lq], _bf16, tag="qs4")
            nc.gpsimd.tensor_tensor(
                qs4[:],
                qnT4,
                inv[:, j * BE : (j + 1) * BE].to_broadcast([D, BE, Lq]),
                op=ALU.mult,
            )
            ps = ps_s.tile([D, Ld], _f32, tag="ps")
            for t in range(BE):
                nc.tensor.matmul(
                    ps[32 * t : 32 * t + 32, :],
                    qs4[:, t * Lq : (t + 1) * Lq],
                    dts[t][:, j * Ld : (j + 1) * Ld],
                    start=True, stop=True,
                    tile_position=(0, 32 * t),
                )
            nc.vector.reduce_max(maxv[:, j : j + 1], ps[:], axis=AX.X)
        sv_ps = ps_q.tile([BE, NDOC], _f32, tag="sv_ps")
        nc.tensor.matmul(sv_ps[:], emat[:], maxv[:], start=True, stop=True)
        nc.vector.tensor_copy(sv_sb[32 * blk : 32 * blk + BE, :], sv_ps[:])

    for blk in range(NBLK):
        emit_norm(blk)
        if blk >= 1:
            emit_groups(blk - 1)
    emit_groups(NBLK - 1)

    # ---- epilogue: log-softmax + KL, rows 32*blk+t ----
    dsc = small.tile([D, N], _f32, tag="dsc")
    nc.vector.tensor_scalar_sub(dsc[:], sv_sb[:, 1:NDOC], sv_sb[:, 0:1])
    mx = tiny.tile([D, 1], _f32, tag="mx")
    nc.vector.reduce_max(mx[:], dsc[:], axis=AX.X)
    nmx = tiny.tile([D, 1], _f32, tag="nmx")
    nc.vector.tensor_scalar_mul(nmx[:], mx[:], -1.0)
    e = small.tile([D, N], _f32, tag="e")
    se = tiny.tile([D, 1], _f32, tag="se")
    nc.scalar.activation(e[:], dsc[:], AF.Exp, bias=nmx[:], accum_out=se[:])
    lse0 = tiny.tile([D, 1], _f32, tag="lse0")
    nc.scalar.activation(lse0[:], se[:], AF.Ln)
    lse = tiny.tile([D, 1], _f32, tag="lse")
    nc.vector.tensor_add(lse[:], lse0[:], mx[:])
    elab = small.tile([D, N], _f32, tag="elab")
    nc.scalar.activation(elab[:], lab_sb[:], AF.Exp)
    t1 = small.tile([D, N], _f32, tag="t1")
    nc.vector.tensor_sub(t1[:], lab_sb[:], dsc[:])
    t2 = small.tile([D, N], _f32, tag="t2")
    nc.vector.tensor_scalar_add(t2[:], t1[:], lse[:])
    t3 = small.tile([D, N], _f32, tag="t3")
    nc.vector.tensor_mul(t3[:], t2[:], elab[:])
    nc.vector.reduce_sum(out_sb[:], t3[:], axis=AX.X)
    nc.sync.dma_start(aps["out"][:], out_sb[:])


def build_program(c_slot):
    nc = bacc.Bacc(
        "TRN2",
        target_bir_lowering=False,
        debug=False,
        enable_asserts=True,
        num_devices=N_CORES,
    )
    aps = {
        "docs": nc.dram_tensor(
            "docs", [PB, D, NDOC * Ld], _bf16, kind="ExternalInput"
        ).ap(),
        "q_t": nc.dram_tensor("q_t", [Lq, PB * D], _bf16, kind="ExternalInput").ap(),
        "ident": nc.dram_tensor("ident", [Lq, Lq], _bf16, kind="ExternalInput").ap(),
        "labels": nc.dram_tensor("labels", [D, N], _f32, kind="ExternalInput").ap(),
        "bncnt": nc.dram_tensor(
            "bncnt", [D, NBLK * 2 * BDOC], _f32, kind="ExternalInput"
        ).ap(),
        "out": nc.dram_tensor("out", [D, 1], _f32, kind="ExternalOutput").ap(),
    }
    with tile.TileContext(nc) as tc:
        with ExitStack() as ctx:
            _emit(ctx, tc, nc, aps, c_slot)
    nc.compile()
    return nc


def shard_inputs(q_emb, pos_emb, neg_emb, labels, pos_mask, neg_mask):
    # docs_all[b, j] = j-th doc of example b (j=0 pos, j>0 neg j-1)
    docs_all = np.empty((B, NDOC, Ld, D), dtype=_np_bf16)
    docs_all[:, 0] = pos_emb.astype(_np_bf16)
    docs_all[:, 1:] = neg_emb.transpose(1, 0, 2, 3).astype(_np_bf16)
    m_all = np.empty((B, NDOC, Ld), dtype=np.int64)
    m_all[:, 0] = pos_mask
    m_all[:, 1:] = neg_mask.transpose(1, 0, 2)

    # unmasked-first token permutation (stable), zero-pad the masked tail
    order = np.argsort(1 - m_all, axis=2, kind="stable")
    c_all = m_all.sum(axis=2)  # [B, NDOC]
    gathered = np.take_along_axis(docs_all, order[..., None], axis=2)
    keep = np.arange(Ld)[None, None, :] < c_all[..., None]
    gathered[~keep] = 0

    # per-slot prefix length = max over the 8 cores (one SPMD program);
    # norm-column order is j-major: u = j*BE + t
    c_by_core = c_all.reshape(N_CORES, PB, NDOC)
    c_slot = []
    for blk in range(NBLK):
        blk_c = c_by_core[:, blk * BE : (blk + 1) * BE, :].max(axis=0)  # [BE, NDOC]
        c_slot.append(tuple(int(blk_c[t, j]) for j in range(NDOC) for t in range(BE)))
    c_slot = tuple(c_slot)

    bncnt = np.zeros((D, NBLK * 2 * BDOC), np.float32)
    for blk in range(NBLK):
        for u in range(BDOC):
            c = c_slot[blk][u]
            bncnt[:, blk * 2 * BDOC + u] = (c + 1) // 2
            bncnt[:, blk * 2 * BDOC + BDOC + u] = c // 2
    lab = np.ascontiguousarray(labels, dtype=np.float32)
    ident = np.eye(Lq, dtype=_np_bf16)
    q_bf = q_emb.astype(_np_bf16)

    in_maps = []
    for cidx in range(N_CORES):
        b0 = cidx * PB
        core_docs = gathered[b0 : b0 + PB]  # [PB, NDOC, Ld, D]
        docs_t = np.ascontiguousarray(
            core_docs.transpose(0, 3, 1, 2).reshape(PB, D, NDOC * Ld)
        )
        in_maps.append(
            {
                "docs": docs_t,
                "q_t": np.ascontiguousarray(
                    q_bf[b0 : b0 + PB].transpose(1, 0, 2).reshape(Lq, PB * D)
                ),
                "ident": ident,
                "labels": _pad_labels(lab[b0 : b0 + PB]),
                "bncnt": bncnt,
            }
        )
    return in_maps, c_slot


def _pad_labels(lab_core):
    out = np.zeros((D, N), np.float32)
    for b in range(PB):
        out[32 * (b // BE) + (b % BE)] = lab_core[b]
    return out


_OUT_ROWS = np.array([32 * (b // BE) + (b % BE) for b in range(PB)])


def kernel(**inputs):
    global _PROGRAM, _PROGRAM_KEY, LAST_RESULTS
    in_maps, c_slot = shard_inputs(
        inputs["q_emb"],
        inputs["pos_emb"],
        inputs["neg_emb"],
        inputs["labels"],
        inputs["pos_mask"],
        inputs["neg_mask"],
    )
    if _PROGRAM is None or _PROGRAM_KEY != c_slot:
        _PROGRAM = build_program(c_slot)
        _PROGRAM_KEY = c_slot
    trace = bool(int(os.environ.get("KBASS_TRACE", "0")))
    res = run_bass_kernel_spmd(_PROGRAM, in_maps, list(range(N_CORES)), trace=trace)
    LAST_RESULTS = res
    parts = np.concatenate(
        [np.asarray(res.results[c]["out"]).reshape(-1)[_OUT_ROWS] for c in range(N_CORES)]
    )
    return np.float32(parts.sum(dtype=np.float64) / B)


# revision 13
# speedup vs baseline: 5.3024x; 1.1331x over previous
"""Trainium2 Bass kernel for MarginKLDivLoss-ColBERT (retrieval maxsim + KL).

Strategy: data-parallel over batch B=128 across 8 NeuronCores (16 examples
per core, 4 blocks of 4 examples each).

Layout prep on host (no FLOPs, pure permutation/cast/packing):
  - Each doc's 512 tokens are PERMUTED so unmasked tokens come first
    (maxsim is permutation-invariant over doc tokens); masked slots are
    zero-padded -- identical to the reference's mask-multiply zeroing.
  - Docs transposed to [D=128, Ld=512] bf16, one contiguous 1.15 MiB DMA
    per example.
  - The per-doc unmasked count c becomes a compile-time prefix length
    (program specialized on the mask pattern; max over cores per slot so
    one SPMD program serves all 8 cores -- shorter docs just read some
    zero padding).

Per core, per 4-example block (36 docs, norm columns in j-major order
u = j*BE + t):
  - ssq[d] = sum_k dT[d,0:c]^2: first K_ACT docs via ACT Square+accum,
    rest via one DVE bn_stats each + a batched moment-recovery
    (ssq = M2e + ne*me^2 + M2o + no*mo^2) over the remaining columns.
  - inv = 1/max(sqrt(ssq),1e-12) batched [128,36].
  - Per group j: ONE batched GPSIMD fold qs4 = qnT(4 examples) * inv
    (free-broadcast), then 4 matmuls into one PSUM [128,512] (quadrants
    0/32/64/96), full 512 cols (zero-padded tails = exact masked zeros).
  - One DVE reduce_max per group: [128,512] -> [128,1] (4 docs at once).
  - sv = E^T @ maxvals (E = 32-block indicator) -> [4 ex, 9 docs].
Groups are emitted one block behind the norm pass so DVE/ACT (norm of
block k+1) overlaps PE/GPSIMD (groups of block k).
Epilogue once per core: log-softmax + KLDiv(log_target) on scattered
rows 32*blk+t; per-example sums DMA'd out; host sums 128 values / B.
"""

import os
import sys
from contextlib import ExitStack

sys.path.insert(0, "/opt/trn_rl_repo")

import ml_dtypes
import numpy as np

import concourse.bass as bass  # noqa: F401  (registers engine classes)
import concourse.bacc as bacc
import concourse.mybir as mybir
import concourse.tile as tile
from concourse.bass_utils import run_bass_kernel_spmd

N_CORES = 8
B, Lq, Ld, D, N = 128, 32, 512, 128, 8
PB = B // N_CORES          # examples per core (16)
NDOC = N + 1               # docs per example (pos + 8 negs)
NBLK = 4                   # blocks per core
BE = PB // NBLK            # examples per block (4)
BDOC = BE * NDOC           # docs per block (36)
K_ACT = 19                 # norm columns [0,K_ACT) per block on ScalarE

_f32 = mybir.dt.float32
_bf16 = mybir.dt.bfloat16
_fp8 = mybir.dt.float8e4
_np_bf16 = ml_dtypes.bfloat16
_np_fp8 = ml_dtypes.float8_e4m3
AF = mybir.ActivationFunctionType
ALU = mybir.AluOpType
AX = mybir.AxisListType

_PROGRAM = None
_PROGRAM_KEY = None
LAST_RESULTS = None


def _emit(ctx, tc, nc, aps, c_slot):
    const = ctx.enter_context(tc.tile_pool(name="const", bufs=1))
    dpool = ctx.enter_context(tc.tile_pool(name="docs", bufs=17))
    qpool = ctx.enter_context(tc.tile_pool(name="qpool", bufs=4))
    qspool = ctx.enter_context(tc.tile_pool(name="qs", bufs=6))
    spool = ctx.enter_context(tc.tile_pool(name="scratch", bufs=4))
    small = ctx.enter_context(tc.tile_pool(name="small", bufs=4))
    tiny = ctx.enter_context(tc.tile_pool(name="tiny", bufs=8))
    ps_s = ctx.enter_context(tc.tile_pool(name="ps_s", bufs=3, space="PSUM"))
    ps_q = ctx.enter_context(tc.tile_pool(name="ps_q", bufs=2, space="PSUM"))

    # ---- constants / whole-core loads ----
    q_sb = const.tile([Lq, PB * D], _bf16)
    nc.sync.dma_start(q_sb[:], aps["q_t"][:])
    ident = const.tile([Lq, Lq], _bf16)
    nc.sync.dma_start(ident[:], aps["ident"][:])
    lab_sb = const.tile([D, N], _f32)
    nc.sync.dma_start(lab_sb[:], aps["labels"][:])
    cnt_sb = const.tile([D, NBLK * 2 * BDOC], _f32)
    nc.sync.dma_start(cnt_sb[:], aps["bncnt"][:])
    emat = const.tile([D, BE], _f32)
    nc.gpsimd.memset(emat[:], 0.0)
    for t in range(BE):
        nc.gpsimd.memset(emat[32 * t : 32 * t + 32, t : t + 1], 1.0)
    qnT_sb = const.tile([D, PB * Lq], _bf16)
    sv_sb = const.tile([D, NDOC], _f32)
    nc.gpsimd.memset(sv_sb[:], 0.0)
    out_sb = const.tile([D, 1], _f32)

    # ---- stage A: q normalization + transpose (all 16 examples) ----
    # ssq_q via DVE bn_stats (c=D=128 -> ne=no=64), batched recovery.
    bnq = const.tile([Lq, PB, 6], _f32)
    for b in range(PB):
        nc.vector.bn_stats(bnq[:, b, :], q_sb[:, b * D : (b + 1) * D])
    ssq_q = const.tile([Lq, PB], _f32)
    rq1 = tiny.tile([Lq, PB], _f32, tag="rq1")
    rq2 = tiny.tile([Lq, PB], _f32, tag="rq2")
    nc.vector.tensor_mul(rq1[:], bnq[:, :, 1], bnq[:, :, 1])
    nc.vector.tensor_mul(rq2[:], bnq[:, :, 4], bnq[:, :, 4])
    nc.vector.tensor_add(rq1[:], rq1[:], rq2[:])
    nc.vector.tensor_scalar_mul(rq1[:], rq1[:], float(D // 2))
    nc.vector.tensor_add(rq2[:], bnq[:, :, 2], bnq[:, :, 5])
    nc.vector.tensor_add(ssq_q[:], rq1[:], rq2[:])
    nrm_q = tiny.tile([Lq, PB], _f32, tag="nrm_q")
    nc.scalar.activation(nrm_q[:], ssq_q[:], AF.Sqrt)
    nrmc_q = tiny.tile([Lq, PB], _f32, tag="nrmc_q")
    nc.vector.tensor_scalar_max(nrmc_q[:], nrm_q[:], 1e-12)
    inv_q = const.tile([Lq, PB], _f32)
    nc.vector.reciprocal(inv_q[:], nrmc_q[:])
    for b in range(PB):
        qb = q_sb[:, b * D : (b + 1) * D]
        qn = qpool.tile([Lq, D], _bf16, tag="qn")
        nc.vector.tensor_scalar_mul(qn[:], qb, inv_q[:, b : b + 1])
        qnT_ps = ps_q.tile([D, Lq], _f32, tag="qnT_ps")
        nc.tensor.matmul(qnT_ps[:], qn[:], ident[:], start=True, stop=True)
        nc.vector.tensor_copy(qnT_sb[:, b * Lq : (b + 1) * Lq], qnT_ps[:])

    # ---- stage B: norm pass per block; groups pipelined 1 block behind ----
    state = {}

    def emit_norm(blk):
        dts = []
        for t in range(BE):
            dt = dpool.tile([D, NDOC * Ld], _fp8, tag="dt")
            nc.sync.dma_start(dt[:], aps["docs"][blk * BE + t])
            dts.append(dt)
        ssq = small.tile([D, BDOC], _f32, tag="ssq")
        wA = spool.tile([D, Ld], _bf16, tag="wA")
        bnt = spool.tile([D, BDOC, 6], _f32, tag="bnt")
        for u in range(BDOC):
            j, t = u // BE, u % BE
            c = c_slot[blk][u]
            if c == 0:
                continue
            seg = dts[t][:, j * Ld : j * Ld + c]
            if u < K_ACT:
                nc.scalar.activation(
                    wA[:, 0:c], seg, AF.Square, accum_out=ssq[:, u : u + 1]
                )
            else:
                nc.vector.bn_stats(bnt[:, u, :], seg)
        # batched bn recovery: ssq[K_ACT:] = M2e+M2o + ne*me^2 + no*mo^2
        nb = BDOC - K_ACT
        ne = cnt_sb[:, blk * 2 * BDOC + K_ACT : blk * 2 * BDOC + BDOC]
        no = cnt_sb[:, blk * 2 * BDOC + BDOC + K_ACT : (blk + 1) * 2 * BDOC]
        r1 = tiny.tile([D, nb], _f32, tag="r1")
        r2 = tiny.tile([D, nb], _f32, tag="r2")
        r3 = tiny.tile([D, nb], _f32, tag="r3")
        nc.vector.tensor_mul(r1[:], bnt[:, K_ACT:BDOC, 1], bnt[:, K_ACT:BDOC, 1])
        nc.vector.tensor_mul(r2[:], r1[:], ne)
        nc.vector.tensor_mul(r1[:], bnt[:, K_ACT:BDOC, 4], bnt[:, K_ACT:BDOC, 4])
        nc.vector.tensor_mul(r3[:], r1[:], no)
        nc.vector.tensor_add(r1[:], bnt[:, K_ACT:BDOC, 2], bnt[:, K_ACT:BDOC, 5])
        nc.vector.tensor_add(r2[:], r2[:], r3[:])
        nc.vector.tensor_add(ssq[:, K_ACT:BDOC], r1[:], r2[:])
        for u in range(BDOC):
            if c_slot[blk][u] == 0:
                nc.gpsimd.memset(ssq[:, u : u + 1], 0.0)
        nrm = tiny.tile([D, BDOC], _f32, tag="nrm")
        nc.scalar.activation(nrm[:], ssq[:], AF.Sqrt)
        nrmc = tiny.tile([D, BDOC], _f32, tag="nrmc")
        nc.vector.tensor_scalar_max(nrmc[:], nrm[:], 1e-12)
        inv = tiny.tile([D, BDOC], _f32, tag="inv")
        nc.vector.reciprocal(inv[:], nrmc[:])
        state[blk] = (dts, inv)

    def emit_groups(blk):
        dts, inv = state.pop(blk)
        qnT4 = qnT_sb[:, blk * BE * Lq : (blk + 1) * BE * Lq]
        maxv = small.tile([D, NDOC], _f32, tag="maxv")
        for j in range(NDOC):
            cg = max(c_slot[blk][j * BE + t] for t in range(BE))
            assert cg < Ld, "c=512 doc needs the full-width reduce"
            qs4 = qspool.tile([D, BE * Lq], _fp8, tag="qs4")
            nc.gpsimd.tensor_tensor(
                qs4[:],
                qnT4,
                inv[:, j * BE : (j + 1) * BE].to_broadcast([D, BE, Lq]),
                op=ALU.mult,
            )
            ps = ps_s.tile([D, Ld], _f32, tag="ps")
            for t in range(BE):
                nc.tensor.matmul(
                    ps[32 * t : 32 * t + 32, 0:cg],
                    qs4[:, t * Lq : (t + 1) * Lq],
                    dts[t][:, j * Ld : j * Ld + cg],
                    start=True, stop=True,
                    tile_position=(0, 32 * t),
                )
            nc.vector.reduce_max(maxv[:, j : j + 1], ps[:, 0:cg], axis=AX.X)
        # masked-token zeros: maxsim includes 0 for every masked slot
        maxvc = small.tile([D, NDOC], _f32, tag="maxvc")
        nc.vector.tensor_scalar_max(maxvc[:], maxv[:], 0.0)
        sv_ps = ps_q.tile([BE, NDOC], _f32, tag="sv_ps")
        nc.tensor.matmul(sv_ps[:], emat[:], maxvc[:], start=True, stop=True)
        nc.vector.tensor_copy(sv_sb[32 * blk : 32 * blk + BE, :], sv_ps[:])

    for blk in range(NBLK):
        emit_norm(blk)
        if blk >= 1:
            emit_groups(blk - 1)
    emit_groups(NBLK - 1)

    # ---- epilogue: log-softmax + KL, rows 32*blk+t ----
    dsc = small.tile([D, N], _f32, tag="dsc")
    nc.vector.tensor_scalar_sub(dsc[:], sv_sb[:, 1:NDOC], sv_sb[:, 0:1])
    mx = tiny.tile([D, 1], _f32, tag="mx")
    nc.vector.reduce_max(mx[:], dsc[:], axis=AX.X)
    nmx = tiny.tile([D, 1], _f32, tag="nmx")
    nc.vector.tensor_scalar_mul(nmx[:], mx[:], -1.0)
    e = small.tile([D, N], _f32, tag="e")
    se = tiny.tile([D, 1], _f32, tag="se")
    nc.scalar.activation(e[:], dsc[:], AF.Exp, bias=nmx[:], accum_out=se[:])
    lse0 = tiny.tile([D, 1], _f32, tag="lse0")
    nc.scalar.activation(lse0[:], se[:], AF.Ln)
    lse = tiny.tile([D, 1], _f32, tag="lse")
    nc.vector.tensor_add(lse[:], lse0[:], mx[:])
    elab = small.tile([D, N], _f32, tag="elab")
    nc.scalar.activation(elab[:], lab_sb[:], AF.Exp)
    t1 = small.tile([D, N], _f32, tag="t1")
    nc.vector.tensor_sub(t1[:], lab_sb[:], dsc[:])
    t2 = small.tile([D, N], _f32, tag="t2")
    nc.vector.tensor_scalar_add(t2[:], t1[:], lse[:])
    t3 = small.tile([D, N], _f32, tag="t3")
    nc.vector.tensor_mul(t3[:], t2[:], elab[:])
    nc.vector.reduce_sum(out_sb[:], t3[:], axis=AX.X)
    nc.sync.dma_start(aps["out"][:], out_sb[:])


def build_program(c_slot):
    nc = bacc.Bacc(
        "TRN2",
        target_bir_lowering=False,
        debug=False,
        enable_asserts=True,
        num_devices=N_CORES,
    )
    aps = {
        "docs": nc.dram_tensor(
            "docs", [PB, D, NDOC * Ld], _fp8, kind="ExternalInput"
        ).ap(),
        "q_t": nc.dram_tensor("q_t", [Lq, PB * D], _bf16, kind="ExternalInput").ap(),
        "ident": nc.dram_tensor("ident", [Lq, Lq], _bf16, kind="ExternalInput").ap(),
        "labels": nc.dram_tensor("labels", [D, N], _f32, kind="ExternalInput").ap(),
        "bncnt": nc.dram_tensor(
            "bncnt", [D, NBLK * 2 * BDOC], _f32, kind="ExternalInput"
        ).ap(),
        "out": nc.dram_tensor("out", [D, 1], _f32, kind="ExternalOutput").ap(),
    }
    with tile.TileContext(nc) as tc:
        with ExitStack() as ctx:
            _emit(ctx, tc, nc, aps, c_slot)
    nc.compile()
    return nc


def shard_inputs(q_emb, pos_emb, neg_emb, labels, pos_mask, neg_mask):
    # docs_all[b, j] = j-th doc of example b (j=0 pos, j>0 neg j-1)
    docs_all = np.empty((B, NDOC, Ld, D), dtype=_np_fp8)
    docs_all[:, 0] = pos_emb.astype(_np_fp8)
    docs_all[:, 1:] = neg_emb.transpose(1, 0, 2, 3).astype(_np_fp8)
    m_all = np.empty((B, NDOC, Ld), dtype=np.int64)
    m_all[:, 0] = pos_mask
    m_all[:, 1:] = neg_mask.transpose(1, 0, 2)

    # unmasked-first token permutation (stable), zero-pad the masked tail
    order = np.argsort(1 - m_all, axis=2, kind="stable")
    c_all = m_all.sum(axis=2)  # [B, NDOC]
    gathered = np.take_along_axis(docs_all, order[..., None], axis=2)
    keep = np.arange(Ld)[None, None, :] < c_all[..., None]
    gathered[~keep] = 0

    # per-slot prefix length = max over the 8 cores (one SPMD program);
    # norm-column order is j-major: u = j*BE + t
    c_by_core = c_all.reshape(N_CORES, PB, NDOC)
    c_slot = []
    for blk in range(NBLK):
        blk_c = c_by_core[:, blk * BE : (blk + 1) * BE, :].max(axis=0)  # [BE, NDOC]
        c_slot.append(tuple(int(blk_c[t, j]) for j in range(NDOC) for t in range(BE)))
    c_slot = tuple(c_slot)

    bncnt = np.zeros((D, NBLK * 2 * BDOC), np.float32)
    for blk in range(NBLK):
        for u in range(BDOC):
            c = c_slot[blk][u]
            bncnt[:, blk * 2 * BDOC + u] = (c + 1) // 2
            bncnt[:, blk * 2 * BDOC + BDOC + u] = c // 2
    lab = np.ascontiguousarray(labels, dtype=np.float32)
    ident = np.eye(Lq, dtype=_np_bf16)
    q_bf = q_emb.astype(_np_bf16)

    in_maps = []
    for cidx in range(N_CORES):
        b0 = cidx * PB
        core_docs = gathered[b0 : b0 + PB]  # [PB, NDOC, Ld, D]
        docs_t = np.ascontiguousarray(
            core_docs.transpose(0, 3, 1, 2).reshape(PB, D, NDOC * Ld)
        )
        in_maps.append(
            {
                "docs": docs_t,
                "q_t": np.ascontiguousarray(
                    q_bf[b0 : b0 + PB].transpose(1, 0, 2).reshape(Lq, PB * D)
                ),
                "ident": ident,
                "labels": _pad_labels(lab[b0 : b0 + PB]),
                "bncnt": bncnt,
            }
        )
    return in_maps, c_slot


def _pad_labels(lab_core):
    out = np.zeros((D, N), np.float32)
    for b in range(PB):
        out[32 * (b // BE) + (b % BE)] = lab_core[b]
    return out


_OUT_ROWS = np.array([32 * (b // BE) + (b % BE) for b in range(PB)])


def kernel(**inputs):
    global _PROGRAM, _PROGRAM_KEY, LAST_RESULTS
    in_maps, c_slot = shard_inputs(
        inputs["q_emb"],
        inputs["pos_emb"],
        inputs["neg_emb"],
        inputs["labels"],
        inputs["pos_mask"],
        inputs["neg_mask"],
    )
    if _PROGRAM is None or _PROGRAM_KEY != c_slot:
        _PROGRAM = build_program(c_slot)
        _PROGRAM_KEY = c_slot
    trace = bool(int(os.environ.get("KBASS_TRACE", "0")))
    res = run_bass_kernel_spmd(_PROGRAM, in_maps, list(range(N_CORES)), trace=trace)
    LAST_RESULTS = res
    parts = np.concatenate(
        [np.asarray(res.results[c]["out"]).reshape(-1)[_OUT_ROWS] for c in range(N_CORES)]
    )
    return np.float32(parts.sum(dtype=np.float64) / B)
